# revision 31
# baseline (speedup 1.0000x reference)
"""MFA E-step kernel for Trainium2 (8 NeuronCores, data-parallel over N).

Math: the reference builds C_k = Lambda_k Lambda_k^T + diag(psi_k) in [K,D,D],
Cholesky-factors it and does a triangular solve per component (~17 GFLOP).
Since C_k is diagonal-plus-rank-Q we use the Woodbury identity and the matrix
determinant lemma instead:

  C^-1 = A^-1 - A^-1 L B^-1 L^T A^-1,  B = I_Q + L^T A^-1 L,  A = diag(a)
  log|C| = log|B| + sum log a

With B = R R^T (Cholesky, [Q,Q]=16x16 - tiny, done on host) and
G = A^-1 L R^-T  [D,Q], the per-sample work reduces to

  log_resp(k, x) = -0.5 * sum_d x^2 inv_a + (inv_a*mu).x + 0.5*||G^T x - g||^2 + C_k

i.e. everything n-dependent is matmuls against X with contraction over D,
plus an elementwise square. All of that accumulates into ONE PSUM tile
[K=32, n] per core on the tensor engine. The non-matmul device steps are the
squares (scalar engine; (t-g)^2 in one instruction via per-partition bias),
a PE transpose into one PSUM bank, and the max/exp/sum of the logsumexp.
The final scalar normalization (log of the [N]-vector of exp-sums and the
broadcast subtract) happens on host during the unshard/gather step, in
float64.

Matmul precision modes (MFA_MODE):
  split (default): bf16 hi/lo 3-term products, A.B ~= Ahi.Bhi+Ahi.Blo+Alo.Bhi
        at 1 cyc/row on the PE (~2x faster than fp32's 4 cyc/row) with
        ~2^-17 per-product error - comfortably inside the fp32 envelope.
  fp32: exact fp32 (walrus lowers to 2 half-speed passes each).
  f32r: single-pass TF32-like (fast but ~2.4e-4 rel error).

Sharding: X is split along N across the 8 cores (512 rows each); the small
component parameters are replicated. No collectives needed.

I/O is packed into few large DMAs per core - the Tile runtime's
end-of-kernel drain scales with DMA queue traffic.
"""

import os

import ml_dtypes
import numpy as np

import concourse.mybir as mybir
import concourse.tile as tile
from concourse import bacc
from concourse.bass_utils import run_bass_kernel_spmd

K, D, Q, N = 32, 256, 16, 4096
N_CORES = 8
NLOC = N // N_CORES          # 512 rows of X per core
KQ = K * Q                   # 512
LOG2PI = float(np.log(2.0 * np.pi))
FP = mybir.dt.float32
BF = mybir.dt.bfloat16

MODE = os.environ.get("MFA_MODE", "split")
assert MODE in ("split", "fp32", "f32r")
MM = mybir.dt.float32r if MODE == "f32r" else mybir.dt.float32
WARMUP_MM = int(os.environ.get("MFA_WARMUP", "2"))

# fp32/f32r packed input: [xt | gs | w12 | cst];  split mode: [xt | cst] fp32
IN_XT = 0
CST_GNEG = 0                  # 4 cols:   -g per kq-tile
CST_ONES = 4                  # 128 cols: 4 blocks of [128, 32] 0.5-valued
CST_ID = CST_ONES + 128       # 32 cols:  identity (rows 0:32)
CST_CK = CST_ID + 32          # 1 col:    per-component constant (rows 0:32)
CST_W = CST_CK + 1            # 165
if MODE == "split":
    IN_CST = NLOC
    IN_W = IN_CST + CST_W
    # bf16 packed input: [gs_hi | gs_lo | w12_hi | w12_lo | ones]
    INB_GSHI = 0
    INB_GSLO = KQ
    INB_W12HI = 2 * KQ
    INB_W12LO = 2 * KQ + 64
    INB_ONES = 2 * KQ + 128
    INB_W = INB_ONES + 128
else:
    IN_GS = NLOC
    IN_W12 = IN_GS + KQ
    IN_CST = IN_W12 + 64
    IN_W = IN_CST + CST_W

OUT_W = K + 2                 # [shifted(32) | -max | sum_exp]


def _fp(ap):
    """View an MM-dtype AP as plain float32 for non-matmul consumers."""
    return ap.bitcast(FP) if MODE == "f32r" else ap


def _build_program():
    nc = bacc.Bacc("TRN2", target_bir_lowering=False)

    inp = nc.dram_tensor("inp", [D, IN_W], MM, kind="ExternalInput")
    if MODE == "split":
        inpb = nc.dram_tensor("inpb", [D, INB_W], BF, kind="ExternalInput")
    out = nc.dram_tensor("out", [NLOC, OUT_W], FP, kind="ExternalOutput")

    with tile.TileContext(nc) as tc:
        with (
            tc.tile_pool(name="data", bufs=1) as dpool,
            tc.tile_pool(name="sq", bufs=1) as spool,
            tc.tile_pool(name="small", bufs=2) as vpool,
            tc.tile_pool(name="warm", bufs=1) as wpool,
            tc.tile_pool(name="ttps", bufs=1 if MODE == "split" else 2,
                         space="PSUM") as ttpool,
            tc.tile_pool(name="rps", bufs=1, space="PSUM") as rpool,
            tc.tile_pool(name="tps", bufs=1, space="PSUM") as tpool,
            tc.tile_pool(name="wps", bufs=1, space="PSUM") as wpspool,
        ):
            # ---- PE warm-up: keep the tensor engine busy through the DMA
            # phase so HAM un-throttles (1.2 -> 2.4 GHz) before real work.
            if WARMUP_MM:
                wsrc = wpool.tile([128, NLOC], FP, tag="wsrc")
                nc.vector.memset(wsrc[:], 0.0)
                wps = wpspool.tile([128, NLOC], FP, tag="wps")
                for i in range(WARMUP_MM):
                    nc.tensor.matmul(wps[:], wsrc[:, 0:128], wsrc[:],
                                     start=(i == 0), stop=(i == WARMUP_MM - 1))

            # ---- loads: one packed DMA per chunk per dram tensor, split
            # across the two HWDGE issuing engines (Sync / Scalar) ----
            # x chunks both on the Sync queue, param chunks both on the
            # Scalar queue: the x->cast->matmul chain then only waits for
            # x transfers, not for whatever else shares the queue.
            in_t = []
            for c in range(2):
                t = dpool.tile([128, IN_W], MM, tag=f"in{c}")
                eng = nc.sync if (c == 0 or MODE == "split") else nc.scalar
                eng.dma_start(t[:], inp[c * 128:(c + 1) * 128, :])
                in_t.append(t)
            xt_t = [in_t[c][:, IN_XT:IN_XT + NLOC] for c in range(2)]
            cst_t = in_t[0][:, IN_CST:IN_CST + CST_W]
            if MODE == "split":
                inb_t = []
                for c in range(2):
                    t = dpool.tile([128, INB_W], BF, tag=f"inb{c}")
                    nc.scalar.dma_start(t[:], inpb[c * 128:(c + 1) * 128, :])
                    inb_t.append(t)
                gs_hi = [inb_t[c][:, INB_GSHI:INB_GSHI + KQ] for c in range(2)]
                gs_lo = [inb_t[c][:, INB_GSLO:INB_GSLO + KQ] for c in range(2)]
                w_hi = [inb_t[c][:, INB_W12HI:INB_W12HI + 64] for c in range(2)]
                w_lo = [inb_t[c][:, INB_W12LO:INB_W12LO + 64] for c in range(2)]
                ones_t = [inb_t[0][:, INB_ONES + 32 * t:INB_ONES + 32 * (t + 1)]
                          for t in range(4)]
            else:
                gs_t = [in_t[c][:, IN_GS:IN_GS + KQ] for c in range(2)]
                w12_t = [in_t[c][:, IN_W12:IN_W12 + 64] for c in range(2)]
                ones_t = [cst_t[:, CST_ONES + 32 * t:CST_ONES + 32 * (t + 1)]
                          for t in range(4)]

            def hi_lo(src_fp, tag, hi_eng=None):
                """bf16 split of a [128, NLOC] fp32 AP: hi = bf16(x) on
                hi_eng (scalar/gpsimd - spreads cast load off the critical
                engine), lo = bf16(x - hi) on the vector engine."""
                hi = spool.tile([128, NLOC], BF, tag=f"{tag}hi")
                if hi_eng is None:
                    nc.scalar.copy(hi[:], src_fp)
                else:
                    hi_eng.tensor_copy(hi[:], src_fp)
                lo = spool.tile([128, NLOC], BF, tag=f"{tag}lo")
                nc.vector.tensor_tensor(lo[:], src_fp, hi[:],
                                        op=mybir.AluOpType.subtract)
                return hi, lo

            # ---- x^2 (scalar engine), bf16 splits (vector engine) ----
            xsq_t, x_s, xsq_s = [], [], []
            for c in range(2):
                if MODE == "split":
                    x_s.append(hi_lo(_fp(xt_t[c]), f"x{c}"))
                xs = spool.tile([128, NLOC], MM, tag=f"xsq{c}")
                nc.scalar.square(xs[:], _fp(xt_t[c]))
                xsq_t.append(xs)
                if MODE == "split":
                    xsq_s.append(hi_lo(_fp(xs[:]), f"xsq{c}", nc.gpsimd))

            # ---- T = G^T X^T  [KQ, NLOC] in 4 partition tiles; S = (T-g)^2 ----
            s_t, s_s = [], []
            if MODE == "split":
                # All 4 tile accumulation groups open at once (4 PSUM banks);
                # terms emitted in operand-readiness order: everything that
                # needs only the hi cast of a chunk before anything needing
                # its lo cast, chunk 0 before chunk 1.
                tts = [ttpool.tile([128, NLOC], FP, tag=f"tt{t}", name=f"tt{t}")
                       for t in range(4)]
                for ci, c in enumerate(range(2)):
                    for cls in range(3):      # 0: ghi.xhi, 1: glo.xhi, 2: ghi.xlo
                        for t in range(4):
                            ghi = gs_hi[c][:, t * 128:(t + 1) * 128]
                            glo = gs_lo[c][:, t * 128:(t + 1) * 128]
                            lh, rh = [(ghi, x_s[c][0][:]), (glo, x_s[c][0][:]),
                                      (ghi, x_s[c][1][:])][cls]
                            nc.tensor.matmul(tts[t][:], lh, rh,
                                             start=(ci == 0 and cls == 0),
                                             stop=(ci == 1 and cls == 2))
                for t in range(4):
                    s = spool.tile([128, NLOC], MM, tag=f"s{t}")
                    nc.scalar.activation(
                        s[:], tts[t][:], mybir.ActivationFunctionType.Square,
                        bias=_fp(cst_t[:, CST_GNEG + t:CST_GNEG + t + 1]),
                        scale=1.0,
                    )
                    s_t.append(s)
                    s_s.append(hi_lo(_fp(s[:]), f"s{t}", nc.gpsimd))
            else:
                for t in range(4):
                    tt = ttpool.tile([128, NLOC], FP, tag="tt")
                    for c in range(2):
                        nc.tensor.matmul(tt[:], gs_t[c][:, t * 128:(t + 1) * 128],
                                         xt_t[c], start=(c == 0), stop=(c == 1))
                    s = spool.tile([128, NLOC], MM, tag=f"s{t}")
                    nc.scalar.activation(
                        s[:], tt[:], mybir.ActivationFunctionType.Square,
                        bias=_fp(cst_t[:, CST_GNEG + t:CST_GNEG + t + 1]),
                        scale=1.0,
                    )
                    s_t.append(s)

            # ---- single PSUM accumulation:  R = -0.5*P + U + 0.5*corr ----
            r_ps = rpool.tile([K, NLOC], FP, tag="r")
            racc = []  # (lhsT, rhs) plain matmuls
            for c in range(2):
                if MODE == "split":
                    racc += [(w_hi[c][:, K:64], x_s[c][0][:]),
                             (w_hi[c][:, K:64], x_s[c][1][:]),
                             (w_lo[c][:, K:64], x_s[c][0][:]),
                             (w_hi[c][:, 0:K], xsq_s[c][0][:]),
                             (w_hi[c][:, 0:K], xsq_s[c][1][:]),
                             (w_lo[c][:, 0:K], xsq_s[c][0][:])]
                else:
                    racc += [(w12_t[c][:, K:64], xt_t[c]),
                             (w12_t[c][:, 0:K], xsq_t[c][:])]
            for t in range(4):
                if MODE == "split":
                    # ones (0.5) is exact in bf16 -> 2-term split suffices
                    racc += [(ones_t[t], s_s[t][0][:]), (ones_t[t], s_s[t][1][:])]
                else:
                    racc.append((ones_t[t], s_t[t][:]))
            for i, (lhsT, rhs) in enumerate(racc):
                nc.tensor.matmul(r_ps[:], lhsT, rhs,
                                 start=(i == 0), stop=(i == len(racc) - 1))

            # ---- log_resps = R + Ck -> SBUF (DVE: fused copy+bias) ----
            rs = spool.tile([K, NLOC], FP, tag="rs")
            nc.vector.tensor_scalar(
                rs[:], r_ps[:], _fp(cst_t[0:K, CST_CK:CST_CK + 1]), None,
                op0=mybir.AluOpType.add,
            )

            # ---- transpose all 4 n-tiles into ONE psum bank [128, 4*K] ----
            tp = tpool.tile([128, 4 * K], FP, tag="tp")
            ident = _fp(cst_t[0:K, CST_ID:CST_ID + K])
            for j in range(4):
                nc.tensor.transpose(
                    tp[:, j * K:(j + 1) * K], rs[:, j * 128:(j + 1) * 128], ident)
            tp3 = tp[:].rearrange("p (j k) -> p j k", k=K)    # [128, 4, K]

            # ---- batched max/exp/sum of the logsumexp; pack one out tile ----
            outt = spool.tile([128, 4 * OUT_W], FP, tag="outt")
            o3 = outt[:].rearrange("p (j k) -> p j k", k=OUT_W)
            negm = o3[:, :, K]                                # [128, 4]
            nc.vector.tensor_reduce(
                o3[:, :, K:K + 1], tp3, axis=mybir.AxisListType.X,
                op=mybir.AluOpType.max, negate=True,
            )
            sh3 = o3[:, :, 0:K]
            nc.vector.tensor_tensor(
                sh3, tp3, negm.broadcast_to([128, 4, K]),
                op=mybir.AluOpType.add,                       # t - max
            )
            e = spool.tile([128, 4 * K], FP, tag="e")
            nc.scalar.activation(
                e[:].rearrange("p (j k) -> p j k", k=K), sh3,
                mybir.ActivationFunctionType.Exp)
            nc.vector.tensor_reduce(
                o3[:, :, K + 1:K + 2], e[:].rearrange("p (j k) -> p j k", k=K),
                axis=mybir.AxisListType.X, op=mybir.AluOpType.add,
            )

            nc.sync.dma_start(
                out.rearrange("(j p) k -> p j k", p=128), o3)

    nc.finalize()
    return nc


_PROGRAM_CACHE = {}


def _get_program():
    if MODE not in _PROGRAM_CACHE:
        _PROGRAM_CACHE[MODE] = _build_program()
    return _PROGRAM_CACHE[MODE]


def _bf_split(A):
    """bf16 (hi, lo) split of a float64 array."""
    hi = A.astype(ml_dtypes.bfloat16)
    lo = (A - hi.astype(np.float64)).astype(ml_dtypes.bfloat16)
    return hi, lo


def _host_prep(X, log_pi, mu, Lambda, log_psi):
    """Tiny O(K*D*Q^2) parameter prep in float64 on host."""
    X = np.asarray(X, np.float64)
    log_pi = np.asarray(log_pi, np.float64)
    mu = np.asarray(mu, np.float64)
    Lam = np.asarray(Lambda, np.float64)
    log_psi = np.asarray(log_psi, np.float64)

    a = np.exp(log_psi) + 1e-6 + 1e-5                     # [K, D]
    inv_a = 1.0 / a
    AL = Lam * inv_a[:, :, None]                          # [K, D, Q]
    B = np.eye(Q)[None] + np.einsum('kdq,kde->kqe', Lam, AL)
    R = np.linalg.cholesky(B)                             # [K, Q, Q]
    logdet = 2.0 * np.sum(np.log(np.diagonal(R, axis1=1, axis2=2)), axis=1) \
        + np.sum(np.log(a), axis=1)                       # [K]
    G = np.linalg.solve(R, AL.transpose(0, 2, 1)).transpose(0, 2, 1)  # [K, D, Q]
    g = np.einsum('kdq,kd->kq', G, mu)                    # [K, Q]
    Ck = log_pi - 0.5 * (D * LOG2PI + logdet + np.sum(mu * mu * inv_a, axis=1))

    f = np.float32
    gsm = G.transpose(1, 0, 2).reshape(D, KQ)             # G as [D, k*Q+q]
    w12 = np.concatenate([-0.5 * inv_a.T, (inv_a * mu).T], axis=1)  # [D, 64]

    cstm = np.zeros((128, CST_W), f)
    # gneg col t, partition p  <-  -g_flat[t*128 + p]  (kq index = k*Q + q)
    cstm[:, CST_GNEG:CST_GNEG + 4] = (-g).reshape(4, 128).T
    onesm = np.zeros((128, 128), f)
    for t in range(4):
        for p in range(128):
            onesm[p, 32 * t + (t * 128 + p) // Q] = 0.5
    cstm[:, CST_ONES:CST_ONES + 128] = onesm
    cstm[0:K, CST_ID:CST_ID + K] = np.eye(K, dtype=f)
    cstm[0:K, CST_CK] = Ck.astype(f)
    xt_full = np.ascontiguousarray(X.T.astype(f))         # [D, N]

    if MODE == "split":
        parb = np.zeros((D, INB_W), ml_dtypes.bfloat16)
        gh, gl = _bf_split(gsm)
        wh, wl = _bf_split(w12)
        parb[:, INB_GSHI:INB_GSHI + KQ] = gh
        parb[:, INB_GSLO:INB_GSLO + KQ] = gl
        parb[:, INB_W12HI:INB_W12HI + 64] = wh
        parb[:, INB_W12LO:INB_W12LO + 64] = wl
        parb[0:128, INB_ONES:INB_ONES + 128] = onesm.astype(ml_dtypes.bfloat16)
        par = cstm                                        # [128, CST_W]
        return xt_full, par, parb
    else:
        par = np.zeros((D, IN_W - NLOC), f)               # [gs | w12 | cst]
        par[:, 0:KQ] = gsm
        par[:, KQ:KQ + 64] = w12
        par[0:128, KQ + 64:] = cstm
        return xt_full, par, None


def make_in_maps(X, log_pi, mu, Lambda, log_psi):
    xt_full, par, parb = _host_prep(X, log_pi, mu, Lambda, log_psi)
    in_maps = []
    for c in range(N_CORES):
        buf = np.zeros((D, IN_W), np.float32)
        buf[:, 0:NLOC] = xt_full[:, c * NLOC:(c + 1) * NLOC]
        if MODE == "split":
            buf[0:128, IN_CST:] = par
        else:
            buf[:, NLOC:] = par
        m = {"inp": buf}
        if MODE == "split":
            m["inpb"] = parb
        in_maps.append(m)
    return in_maps


def finish_outputs(results):
    """Gather per-core outputs; final scalar normalization in float64."""
    raw = np.concatenate([r["out"] for r in results], axis=0)  # [N, K+2]
    shifted = raw[:, 0:K].astype(np.float64)
    negm = raw[:, K].astype(np.float64)
    ssum = raw[:, K + 1].astype(np.float64)
    lse = np.log(ssum)                                    # [N]
    resp = (shifted - lse[:, None]).astype(np.float32)    # log_resp_norm [N, K]
    ll = (lse - negm).astype(np.float32)                  # log_likelihood [N]
    return resp, ll


def kernel(X, log_pi, mu, Lambda, log_psi):
    nc = _get_program()
    in_maps = make_in_maps(X, log_pi, mu, Lambda, log_psi)
    res = run_bass_kernel_spmd(nc, in_maps, core_ids=list(range(N_CORES)))

    return finish_outputs(res.results)


if __name__ == "__main__":
    rng = np.random.default_rng(0)
    inputs = {
        "X": rng.standard_normal((N, D)).astype(np.float32),
        "log_pi": np.full((K,), -np.log(K), np.float32),
        "mu": (0.1 * rng.standard_normal((K, D))).astype(np.float32),
        "Lambda": (0.1 * rng.standard_normal((K, D, Q))).astype(np.float32),
        "log_psi": (np.log(0.01) + 0.1 * rng.standard_normal((K, D))).astype(np.float32),
    }
    resp, ll = kernel(**inputs)
    print("resp", resp.shape, resp.dtype, "ll", ll.shape, ll.dtype)


# revision 38
# speedup vs baseline: 1.2830x; 1.2830x over previous
"""MFA E-step kernel for Trainium2 (8 NeuronCores, data-parallel over N).

Math: the reference builds C_k = Lambda_k Lambda_k^T + diag(psi_k) in [K,D,D],
Cholesky-factors it and does a triangular solve per component (~17 GFLOP).
Since C_k is diagonal-plus-rank-Q we use the Woodbury identity and the matrix
determinant lemma instead:

  C^-1 = A^-1 - A^-1 L B^-1 L^T A^-1,  B = I_Q + L^T A^-1 L,  A = diag(a)
  log|C| = log|B| + sum log a

With B = R R^T (Cholesky, [Q,Q]=16x16 - tiny, done on host) and
G = A^-1 L R^-T  [D,Q], the per-sample work reduces to

  log_resp(k, x) = -0.5 * sum_d x^2 inv_a + (inv_a*mu).x + 0.5*||G^T x - g||^2 + C_k

i.e. everything n-dependent is matmuls against X with contraction over D,
plus an elementwise square. All of that accumulates into ONE PSUM tile
[K=32, n] per core on the tensor engine. The non-matmul device steps are the
squares (scalar engine; (t-g)^2 in one instruction via per-partition bias),
a PE transpose into one PSUM bank, and the max/exp/sum of the logsumexp.
The final scalar normalization (log of the [N]-vector of exp-sums and the
broadcast subtract) happens on host during the unshard/gather step, in
float64.

Matmul precision modes (MFA_MODE):
  split (default): bf16 hi/lo 3-term products, A.B ~= Ahi.Bhi+Ahi.Blo+Alo.Bhi
        at 1 cyc/row on the PE (~2x faster than fp32's 4 cyc/row) with
        ~2^-17 per-product error - comfortably inside the fp32 envelope.
  fp32: exact fp32 (walrus lowers to 2 half-speed passes each).
  f32r: single-pass TF32-like (fast but ~2.4e-4 rel error).

Sharding: X is split along N across the 8 cores (512 rows each); the small
component parameters are replicated. No collectives needed.

I/O is packed into few large DMAs per core - the Tile runtime's
end-of-kernel drain scales with DMA queue traffic.
"""

import os

import ml_dtypes
import numpy as np

import concourse.mybir as mybir
import concourse.tile as tile
from concourse import bacc
from concourse.bass_utils import run_bass_kernel_spmd

K, D, Q, N = 32, 256, 16, 4096
N_CORES = 8
NLOC = N // N_CORES          # 512 rows of X per core
KQ = K * Q                   # 512
LOG2PI = float(np.log(2.0 * np.pi))
FP = mybir.dt.float32
BF = mybir.dt.bfloat16

MODE = os.environ.get("MFA_MODE", "split")
assert MODE in ("split", "fp32", "f32r")
MM = mybir.dt.float32r if MODE == "f32r" else mybir.dt.float32
WARMUP_MM = int(os.environ.get("MFA_WARMUP", "2"))

# fp32/f32r packed input: [xt | gs | w12 | cst];  split mode: [xt | cst] fp32
IN_XT = 0
CST_GNEG = 0                  # 4 cols:   -g per kq-tile
CST_ONES = 4                  # 128 cols: 4 blocks of [128, 32] 0.5-valued
CST_ID = CST_ONES + 128       # 32 cols:  identity (rows 0:32)
CST_CK = CST_ID + 32          # 1 col:    per-component constant (rows 0:32)
CST_W = CST_CK + 1            # 165
if MODE == "split":
    IN_CST = NLOC
    IN_W = IN_CST + CST_W
    # bf16 packed input: [gs_hi | gs_lo | w2pack | w1pack | ones] where
    # w2pack = [W2hi|W2lo], w1pack = [W1hi|W1lo] (64-wide stationaries; the
    # hi/lo output row-halves of the R psum get summed in the rs step)
    INB_GSHI = 0
    INB_GSLO = KQ
    INB_W2P = 2 * KQ
    INB_W1P = 2 * KQ + 64
    INB_ONES = 2 * KQ + 128
    INB_W = INB_ONES + 128
else:
    IN_GS = NLOC
    IN_W12 = IN_GS + KQ
    IN_CST = IN_W12 + 64
    IN_W = IN_CST + CST_W

OUT_W = K + 2                 # [shifted(32) | -max | sum_exp]


def _fp(ap):
    """View an MM-dtype AP as plain float32 for non-matmul consumers."""
    return ap.bitcast(FP) if MODE == "f32r" else ap


def _build_program():
    nc = bacc.Bacc("TRN2", target_bir_lowering=False)

    inp = nc.dram_tensor("inp", [D, IN_W], MM, kind="ExternalInput")
    if MODE == "split":
        inpb = nc.dram_tensor("inpb", [D, INB_W], BF, kind="ExternalInput")
    out = nc.dram_tensor("out", [NLOC, OUT_W], FP, kind="ExternalOutput")

    with tile.TileContext(nc) as tc:
        with (
            tc.tile_pool(name="data", bufs=1) as dpool,
            tc.tile_pool(name="sq", bufs=1) as spool,
            tc.tile_pool(name="small", bufs=2) as vpool,
            tc.tile_pool(name="warm", bufs=1) as wpool,
            tc.tile_pool(name="ttps", bufs=1 if MODE == "split" else 2,
                         space="PSUM") as ttpool,
            tc.tile_pool(name="rps", bufs=1, space="PSUM") as rpool,
            tc.tile_pool(name="tps", bufs=1, space="PSUM") as tpool,
            tc.tile_pool(name="wps", bufs=1, space="PSUM") as wpspool,
        ):
            # ---- PE warm-up: keep the tensor engine busy through the DMA
            # phase so HAM un-throttles (1.2 -> 2.4 GHz) before real work.
            if WARMUP_MM:
                wsrc = wpool.tile([128, NLOC], FP, tag="wsrc")
                nc.vector.memset(wsrc[:], 0.0)
                wps = wpspool.tile([128, NLOC], FP, tag="wps")
                for i in range(WARMUP_MM):
                    nc.tensor.matmul(wps[:], wsrc[:, 0:128], wsrc[:],
                                     start=(i == 0), stop=(i == WARMUP_MM - 1))

            # ---- loads: one packed DMA per chunk per dram tensor, split
            # across the two HWDGE issuing engines (Sync / Scalar) ----
            # x chunks both on the Sync queue, param chunks both on the
            # Scalar queue: the x->cast->matmul chain then only waits for
            # x transfers, not for whatever else shares the queue.
            in_t = []
            for c in range(2):
                t = dpool.tile([128, IN_W], MM, tag=f"in{c}")
                eng = nc.sync if (c == 0 or MODE == "split") else nc.scalar
                eng.dma_start(t[:], inp[c * 128:(c + 1) * 128, :])
                in_t.append(t)
            xt_t = [in_t[c][:, IN_XT:IN_XT + NLOC] for c in range(2)]
            cst_t = in_t[0][:, IN_CST:IN_CST + CST_W]
            if MODE == "split":
                inb_t = []
                for c in range(2):
                    t = dpool.tile([128, INB_W], BF, tag=f"inb{c}")
                    nc.scalar.dma_start(t[:], inpb[c * 128:(c + 1) * 128, :])
                    inb_t.append(t)
                gs_hi = [inb_t[c][:, INB_GSHI:INB_GSHI + KQ] for c in range(2)]
                gs_lo = [inb_t[c][:, INB_GSLO:INB_GSLO + KQ] for c in range(2)]
                w2p = [inb_t[c][:, INB_W2P:INB_W2P + 64] for c in range(2)]
                w1p = [inb_t[c][:, INB_W1P:INB_W1P + 64] for c in range(2)]
                ones_t = [inb_t[0][:, INB_ONES + 32 * t:INB_ONES + 32 * (t + 1)]
                          for t in range(4)]
            else:
                gs_t = [in_t[c][:, IN_GS:IN_GS + KQ] for c in range(2)]
                w12_t = [in_t[c][:, IN_W12:IN_W12 + 64] for c in range(2)]
                ones_t = [cst_t[:, CST_ONES + 32 * t:CST_ONES + 32 * (t + 1)]
                          for t in range(4)]

            def hi_lo(src_fp, tag, hi_eng=None):
                """bf16 split of a [128, NLOC] fp32 AP: hi = bf16(x) on
                hi_eng (scalar/gpsimd - spreads cast load off the critical
                engine), lo = bf16(x - hi) on the vector engine."""
                hi = spool.tile([128, NLOC], BF, tag=f"{tag}hi")
                if hi_eng is None:
                    nc.scalar.copy(hi[:], src_fp)
                else:
                    hi_eng.tensor_copy(hi[:], src_fp)
                lo = spool.tile([128, NLOC], BF, tag=f"{tag}lo")
                nc.vector.tensor_tensor(lo[:], src_fp, hi[:],
                                        op=mybir.AluOpType.subtract)
                return hi, lo

            # ---- x^2 (scalar engine), bf16 splits (vector engine) ----
            xsq_t, x_s, xsq_s = [], [], []
            for c in range(2):
                if MODE == "split":
                    x_s.append(hi_lo(_fp(xt_t[c]), f"x{c}"))
                xs = spool.tile([128, NLOC], MM, tag=f"xsq{c}")
                nc.scalar.square(xs[:], _fp(xt_t[c]))
                xsq_t.append(xs)
                if MODE == "split":
                    xsq_s.append(hi_lo(_fp(xs[:]), f"xsq{c}", nc.vector))

            # ---- T = G^T X^T  [KQ, NLOC] in 4 partition tiles; S = (T-g)^2 ----
            s_t, s_s = [], []
            if MODE == "split":
                # All 4 tile accumulation groups open at once (4 PSUM banks);
                # terms emitted in operand-readiness order: everything that
                # needs only the hi cast of a chunk before anything needing
                # its lo cast, chunk 0 before chunk 1.
                tts = [ttpool.tile([128, NLOC], FP, tag=f"tt{t}", name=f"tt{t}")
                       for t in range(4)]
                for ci, c in enumerate(range(2)):
                    for cls in range(3):      # 0: ghi.xhi, 1: glo.xhi, 2: ghi.xlo
                        for t in range(4):
                            ghi = gs_hi[c][:, t * 128:(t + 1) * 128]
                            glo = gs_lo[c][:, t * 128:(t + 1) * 128]
                            lh, rh = [(ghi, x_s[c][0][:]), (glo, x_s[c][0][:]),
                                      (ghi, x_s[c][1][:])][cls]
                            nc.tensor.matmul(tts[t][:], lh, rh,
                                             start=(ci == 0 and cls == 0),
                                             stop=(ci == 1 and cls == 2))
                for t in range(4):
                    s = spool.tile([128, NLOC], MM, tag=f"s{t}")
                    nc.scalar.activation(
                        s[:], tts[t][:], mybir.ActivationFunctionType.Square,
                        bias=_fp(cst_t[:, CST_GNEG + t:CST_GNEG + t + 1]),
                        scale=1.0,
                    )
                    s_t.append(s)
                    s_s.append(hi_lo(_fp(s[:]), f"s{t}", nc.vector))
            else:
                for t in range(4):
                    tt = ttpool.tile([128, NLOC], FP, tag="tt")
                    for c in range(2):
                        nc.tensor.matmul(tt[:], gs_t[c][:, t * 128:(t + 1) * 128],
                                         xt_t[c], start=(c == 0), stop=(c == 1))
                    s = spool.tile([128, NLOC], MM, tag=f"s{t}")
                    nc.scalar.activation(
                        s[:], tt[:], mybir.ActivationFunctionType.Square,
                        bias=_fp(cst_t[:, CST_GNEG + t:CST_GNEG + t + 1]),
                        scale=1.0,
                    )
                    s_t.append(s)

            # ---- single PSUM accumulation:  R = -0.5*P + U + 0.5*corr ----
            # split mode: 64-row psum; the hi-stationary terms land in rows
            # 0:32 and the lo-stationary terms in rows 32:64 (64-wide packed
            # stationaries make each 512-col moving pass do double duty);
            # the rs step sums the halves.
            rs = spool.tile([K, NLOC], FP, tag="rs")
            if MODE == "split":
                r_ps = rpool.tile([K, NLOC], FP, tag="r")
                racc = []  # (lhsT, rhs)
                for c in range(2):
                    racc += [(w2p[c][:, 0:K], x_s[c][0][:]),
                             (w2p[c][:, 0:K], x_s[c][1][:]),
                             (w2p[c][:, K:64], x_s[c][0][:]),
                             (w1p[c][:, 0:K], xsq_s[c][0][:]),
                             (w1p[c][:, 0:K], xsq_s[c][1][:]),
                             (w1p[c][:, K:64], xsq_s[c][0][:])]
                for t in range(4):
                    # ones (0.5) is exact in bf16 -> 2-term split suffices
                    racc += [(ones_t[t], s_s[t][0][:]),
                             (ones_t[t], s_s[t][1][:])]
                for i, (lhsT, rhs) in enumerate(racc):
                    nc.tensor.matmul(r_ps[:], lhsT, rhs,
                                     start=(i == 0), stop=(i == len(racc) - 1))
                nc.vector.tensor_scalar(
                    rs[:], r_ps[:], _fp(cst_t[0:K, CST_CK:CST_CK + 1]), None,
                    op0=mybir.AluOpType.add,
                )
            else:
                r_ps = rpool.tile([K, NLOC], FP, tag="r")
                racc = []
                for c in range(2):
                    racc += [(w12_t[c][:, K:64], xt_t[c]),
                             (w12_t[c][:, 0:K], xsq_t[c][:])]
                for t in range(4):
                    racc.append((ones_t[t], s_t[t][:]))
                for i, (lhsT, rhs) in enumerate(racc):
                    nc.tensor.matmul(r_ps[:], lhsT, rhs,
                                     start=(i == 0), stop=(i == len(racc) - 1))
                nc.vector.tensor_scalar(
                    rs[:], r_ps[:], _fp(cst_t[0:K, CST_CK:CST_CK + 1]), None,
                    op0=mybir.AluOpType.add,
                )

            # ---- transpose all 4 n-tiles into ONE psum bank [128, 4*K] ----
            tp = tpool.tile([128, 4 * K], FP, tag="tp")
            ident = _fp(cst_t[0:K, CST_ID:CST_ID + K])
            for j in range(4):
                nc.tensor.transpose(
                    tp[:, j * K:(j + 1) * K], rs[:, j * 128:(j + 1) * 128], ident)
            tp3 = tp[:].rearrange("p (j k) -> p j k", k=K)    # [128, 4, K]

            # ---- batched max/exp/sum of the logsumexp; pack one out tile ----
            outt = spool.tile([128, 4 * OUT_W], FP, tag="outt")
            o3 = outt[:].rearrange("p (j k) -> p j k", k=OUT_W)
            negm = o3[:, :, K]                                # [128, 4]
            nc.vector.tensor_reduce(
                o3[:, :, K:K + 1], tp3, axis=mybir.AxisListType.X,
                op=mybir.AluOpType.max, negate=True,
            )
            sh3 = o3[:, :, 0:K]
            nc.vector.tensor_tensor(
                sh3, tp3, negm.broadcast_to([128, 4, K]),
                op=mybir.AluOpType.add,                       # t - max
            )
            e = spool.tile([128, 4 * K], FP, tag="e")
            nc.scalar.activation(
                e[:].rearrange("p (j k) -> p j k", k=K), sh3,
                mybir.ActivationFunctionType.Exp)
            nc.vector.tensor_reduce(
                o3[:, :, K + 1:K + 2], e[:].rearrange("p (j k) -> p j k", k=K),
                axis=mybir.AxisListType.X, op=mybir.AluOpType.add,
            )

            nc.sync.dma_start(
                out.rearrange("(j p) k -> p j k", p=128), o3)

    nc.finalize()
    return nc


_PROGRAM_CACHE = {}


def _get_program():
    if MODE not in _PROGRAM_CACHE:
        _PROGRAM_CACHE[MODE] = _build_program()
    return _PROGRAM_CACHE[MODE]


def _bf_split(A):
    """bf16 (hi, lo) split of a float64 array."""
    hi = A.astype(ml_dtypes.bfloat16)
    lo = (A - hi.astype(np.float64)).astype(ml_dtypes.bfloat16)
    return hi, lo


def _host_prep(X, log_pi, mu, Lambda, log_psi):
    """Tiny O(K*D*Q^2) parameter prep in float64 on host."""
    X = np.asarray(X, np.float64)
    log_pi = np.asarray(log_pi, np.float64)
    mu = np.asarray(mu, np.float64)
    Lam = np.asarray(Lambda, np.float64)
    log_psi = np.asarray(log_psi, np.float64)

    a = np.exp(log_psi) + 1e-6 + 1e-5                     # [K, D]
    inv_a = 1.0 / a
    AL = Lam * inv_a[:, :, None]                          # [K, D, Q]
    B = np.eye(Q)[None] + np.einsum('kdq,kde->kqe', Lam, AL)
    R = np.linalg.cholesky(B)                             # [K, Q, Q]
    logdet = 2.0 * np.sum(np.log(np.diagonal(R, axis1=1, axis2=2)), axis=1) \
        + np.sum(np.log(a), axis=1)                       # [K]
    G = np.linalg.solve(R, AL.transpose(0, 2, 1)).transpose(0, 2, 1)  # [K, D, Q]
    g = np.einsum('kdq,kd->kq', G, mu)                    # [K, Q]
    Ck = log_pi - 0.5 * (D * LOG2PI + logdet + np.sum(mu * mu * inv_a, axis=1))

    f = np.float32
    gsm = G.transpose(1, 0, 2).reshape(D, KQ)             # G as [D, k*Q+q]
    w12 = np.concatenate([-0.5 * inv_a.T, (inv_a * mu).T], axis=1)  # [D, 64]

    cstm = np.zeros((128, CST_W), f)
    # gneg col t, partition p  <-  -g_flat[t*128 + p]  (kq index = k*Q + q)
    cstm[:, CST_GNEG:CST_GNEG + 4] = (-g).reshape(4, 128).T
    onesm = np.zeros((128, 128), f)
    for t in range(4):
        for p in range(128):
            onesm[p, 32 * t + (t * 128 + p) // Q] = 0.5
    cstm[:, CST_ONES:CST_ONES + 128] = onesm
    cstm[0:K, CST_ID:CST_ID + K] = np.eye(K, dtype=f)
    cstm[0:K, CST_CK] = Ck.astype(f)
    xt_full = np.ascontiguousarray(X.T.astype(f))         # [D, N]

    if MODE == "split":
        parb = np.zeros((D, INB_W), ml_dtypes.bfloat16)
        gh, gl = _bf_split(gsm)
        wh, wl = _bf_split(w12)
        parb[:, INB_GSHI:INB_GSHI + KQ] = gh
        parb[:, INB_GSLO:INB_GSLO + KQ] = gl
        # w2pack = [W2hi|W2lo], w1pack = [W1hi|W1lo]  (w12 = [W1 | W2])
        parb[:, INB_W2P:INB_W2P + K] = wh[:, K:64]
        parb[:, INB_W2P + K:INB_W2P + 64] = wl[:, K:64]
        parb[:, INB_W1P:INB_W1P + K] = wh[:, 0:K]
        parb[:, INB_W1P + K:INB_W1P + 64] = wl[:, 0:K]
        parb[0:128, INB_ONES:INB_ONES + 128] = onesm.astype(ml_dtypes.bfloat16)
        par = cstm                                        # [128, CST_W]
        return xt_full, par, parb
    else:
        par = np.zeros((D, IN_W - NLOC), f)               # [gs | w12 | cst]
        par[:, 0:KQ] = gsm
        par[:, KQ:KQ + 64] = w12
        par[0:128, KQ + 64:] = cstm
        return xt_full, par, None


def make_in_maps(X, log_pi, mu, Lambda, log_psi):
    xt_full, par, parb = _host_prep(X, log_pi, mu, Lambda, log_psi)
    in_maps = []
    for c in range(N_CORES):
        buf = np.zeros((D, IN_W), np.float32)
        buf[:, 0:NLOC] = xt_full[:, c * NLOC:(c + 1) * NLOC]
        if MODE == "split":
            buf[0:128, IN_CST:] = par
        else:
            buf[:, NLOC:] = par
        m = {"inp": buf}
        if MODE == "split":
            m["inpb"] = parb
        in_maps.append(m)
    return in_maps


def finish_outputs(results):
    """Gather per-core outputs; final scalar normalization in float64."""
    raw = np.concatenate([r["out"] for r in results], axis=0)  # [N, K+2]
    shifted = raw[:, 0:K].astype(np.float64)
    negm = raw[:, K].astype(np.float64)
    ssum = raw[:, K + 1].astype(np.float64)
    lse = np.log(ssum)                                    # [N]
    resp = (shifted - lse[:, None]).astype(np.float32)    # log_resp_norm [N, K]
    ll = (lse - negm).astype(np.float32)                  # log_likelihood [N]
    return resp, ll


def kernel(X, log_pi, mu, Lambda, log_psi):
    nc = _get_program()
    in_maps = make_in_maps(X, log_pi, mu, Lambda, log_psi)
    res = run_bass_kernel_spmd(nc, in_maps, core_ids=list(range(N_CORES)))

    return finish_outputs(res.results)


if __name__ == "__main__":
    rng = np.random.default_rng(0)
    inputs = {
        "X": rng.standard_normal((N, D)).astype(np.float32),
        "log_pi": np.full((K,), -np.log(K), np.float32),
        "mu": (0.1 * rng.standard_normal((K, D))).astype(np.float32),
        "Lambda": (0.1 * rng.standard_normal((K, D, Q))).astype(np.float32),
        "log_psi": (np.log(0.01) + 0.1 * rng.standard_normal((K, D))).astype(np.float32),
    }
    resp, ll = kernel(**inputs)
    print("resp", resp.shape, resp.dtype, "ll", ll.shape, ll.dtype)


# revision 42
# speedup vs baseline: 1.3498x; 1.0520x over previous
"""MFA E-step kernel for Trainium2 (8 NeuronCores, data-parallel over N).

Math: the reference builds C_k = Lambda_k Lambda_k^T + diag(psi_k) in [K,D,D],
Cholesky-factors it and does a triangular solve per component (~17 GFLOP).
Since C_k is diagonal-plus-rank-Q we use the Woodbury identity and the matrix
determinant lemma instead:

  C^-1 = A^-1 - A^-1 L B^-1 L^T A^-1,  B = I_Q + L^T A^-1 L,  A = diag(a)
  log|C| = log|B| + sum log a

With B = R R^T (Cholesky, [Q,Q]=16x16 - tiny, done on host) and
G = A^-1 L R^-T  [D,Q], the per-sample work reduces to

  log_resp(k, x) = -0.5 * sum_d x^2 inv_a + (inv_a*mu).x + 0.5*||G^T x - g||^2 + C_k

i.e. everything n-dependent is matmuls against X with contraction over D,
plus an elementwise square. All of that accumulates into ONE PSUM tile
[K=32, n] per core on the tensor engine. The non-matmul device steps are the
squares (scalar engine; (t-g)^2 in one instruction via per-partition bias),
a PE transpose into one PSUM bank, and the max/exp/sum of the logsumexp.
The final scalar normalization (log of the [N]-vector of exp-sums and the
broadcast subtract) happens on host during the unshard/gather step, in
float64.

Matmul precision modes (MFA_MODE):
  split (default): bf16 hi/lo 3-term products, A.B ~= Ahi.Bhi+Ahi.Blo+Alo.Bhi
        at 1 cyc/row on the PE (~2x faster than fp32's 4 cyc/row) with
        ~2^-17 per-product error - comfortably inside the fp32 envelope.
  fp32: exact fp32 (walrus lowers to 2 half-speed passes each).
  f32r: single-pass TF32-like (fast but ~2.4e-4 rel error).

Sharding: X is split along N across the 8 cores (512 rows each); the small
component parameters are replicated. No collectives needed.

I/O is packed into few large DMAs per core - the Tile runtime's
end-of-kernel drain scales with DMA queue traffic.
"""

import os

import ml_dtypes
import numpy as np

import concourse.mybir as mybir
import concourse.tile as tile
from concourse import bacc
from concourse.bass_utils import run_bass_kernel_spmd

K, D, Q, N = 32, 256, 16, 4096
N_CORES = 8
NLOC = N // N_CORES          # 512 rows of X per core
KQ = K * Q                   # 512
LOG2PI = float(np.log(2.0 * np.pi))
FP = mybir.dt.float32
BF = mybir.dt.bfloat16

MODE = os.environ.get("MFA_MODE", "split")
assert MODE in ("split", "fp32", "f32r")
MM = mybir.dt.float32r if MODE == "f32r" else mybir.dt.float32
WARMUP_MM = int(os.environ.get("MFA_WARMUP", "2"))

# fp32/f32r packed input: [xt | gs | w12 | cst];  split mode: [xt | cst] fp32
IN_XT = 0
CST_GNEG = 0                  # 4 cols:   -g per kq-tile
CST_ONES = 4                  # 128 cols: 4 blocks of [128, 32] 0.5-valued
CST_ID = CST_ONES + 128       # 32 cols:  identity (rows 0:32)
CST_CK = CST_ID + 32          # 1 col:    per-component constant (rows 0:32)
CST_W = CST_CK + 1            # 165
if MODE == "split":
    IN_CST = NLOC
    # bf16 param block [gs_hi | gs_lo | w2pack | w1pack | ones] is packed
    # INSIDE the fp32 input tensor (2 bf16 per fp32 slot, bitcast on SBUF)
    # so each chunk is a single DMA - the Tile end-of-kernel drain scales
    # with DMA count.  w2pack = [W2hi|W2lo], w1pack = [W1hi|W1lo].
    INB_GSHI = 0
    INB_GSLO = KQ
    INB_W2P = 2 * KQ
    INB_W1P = 2 * KQ + 64
    INB_ONES = 2 * KQ + 128
    INB_W = INB_ONES + 128            # 1280 bf16 = 640 fp32 cols
    IN_BF = IN_CST + CST_W            # fp32 col where the bf16 block starts
    IN_W = IN_BF + INB_W // 2
else:
    IN_GS = NLOC
    IN_W12 = IN_GS + KQ
    IN_CST = IN_W12 + 64
    IN_W = IN_CST + CST_W

OUT_W = K + 2                 # [shifted(32) | -max | sum_exp]


def _fp(ap):
    """View an MM-dtype AP as plain float32 for non-matmul consumers."""
    return ap.bitcast(FP) if MODE == "f32r" else ap


def _build_program():
    nc = bacc.Bacc("TRN2", target_bir_lowering=False)

    inp = nc.dram_tensor("inp", [D, IN_W], MM, kind="ExternalInput")
    out = nc.dram_tensor("out", [NLOC, OUT_W], FP, kind="ExternalOutput")

    with tile.TileContext(nc) as tc:
        with (
            tc.tile_pool(name="data", bufs=1) as dpool,
            tc.tile_pool(name="sq", bufs=1) as spool,
            tc.tile_pool(name="small", bufs=2) as vpool,
            tc.tile_pool(name="warm", bufs=1) as wpool,
            tc.tile_pool(name="ttps", bufs=1 if MODE == "split" else 2,
                         space="PSUM") as ttpool,
            tc.tile_pool(name="rps", bufs=1, space="PSUM") as rpool,
            tc.tile_pool(name="tps", bufs=1, space="PSUM") as tpool,
            tc.tile_pool(name="wps", bufs=1, space="PSUM") as wpspool,
        ):
            # ---- PE warm-up: keep the tensor engine busy through the DMA
            # phase so HAM un-throttles (1.2 -> 2.4 GHz) before real work.
            if WARMUP_MM:
                wsrc = wpool.tile([128, NLOC], FP, tag="wsrc")
                nc.vector.memset(wsrc[:], 0.0)
                wps = wpspool.tile([128, NLOC], FP, tag="wps")
                for i in range(WARMUP_MM):
                    nc.tensor.matmul(wps[:], wsrc[:, 0:128], wsrc[:],
                                     start=(i == 0), stop=(i == WARMUP_MM - 1))

            # ---- loads: one packed DMA per chunk per dram tensor, split
            # across the two HWDGE issuing engines (Sync / Scalar) ----
            # one packed DMA per chunk, one per HWDGE issuing engine
            in_t = []
            for c in range(2):
                t = dpool.tile([128, IN_W], MM, tag=f"in{c}")
                eng = nc.sync if c == 0 else nc.scalar
                eng.dma_start(t[:], inp[c * 128:(c + 1) * 128, :])
                in_t.append(t)
            xt_t = [in_t[c][:, IN_XT:IN_XT + NLOC] for c in range(2)]
            cst_t = in_t[0][:, IN_CST:IN_CST + CST_W]
            if MODE == "split":
                inb_t = [in_t[c][:, IN_BF:IN_BF + INB_W // 2].bitcast(BF)
                         for c in range(2)]
                gs_hi = [inb_t[c][:, INB_GSHI:INB_GSHI + KQ] for c in range(2)]
                gs_lo = [inb_t[c][:, INB_GSLO:INB_GSLO + KQ] for c in range(2)]
                w2p = [inb_t[c][:, INB_W2P:INB_W2P + 64] for c in range(2)]
                w1p = [inb_t[c][:, INB_W1P:INB_W1P + 64] for c in range(2)]
                ones_t = [inb_t[0][:, INB_ONES + 32 * t:INB_ONES + 32 * (t + 1)]
                          for t in range(4)]
            else:
                gs_t = [in_t[c][:, IN_GS:IN_GS + KQ] for c in range(2)]
                w12_t = [in_t[c][:, IN_W12:IN_W12 + 64] for c in range(2)]
                ones_t = [cst_t[:, CST_ONES + 32 * t:CST_ONES + 32 * (t + 1)]
                          for t in range(4)]

            def hi_lo(src_fp, tag, hi_eng=None):
                """bf16 split of a [128, NLOC] fp32 AP: hi = bf16(x) on
                hi_eng (scalar/gpsimd - spreads cast load off the critical
                engine), lo = bf16(x - hi) on the vector engine."""
                hi = spool.tile([128, NLOC], BF, tag=f"{tag}hi")
                if hi_eng is None:
                    nc.scalar.copy(hi[:], src_fp)
                else:
                    hi_eng.tensor_copy(hi[:], src_fp)
                lo = spool.tile([128, NLOC], BF, tag=f"{tag}lo")
                nc.vector.tensor_tensor(lo[:], src_fp, hi[:],
                                        op=mybir.AluOpType.subtract)
                return hi, lo

            # ---- x^2 (scalar engine), bf16 splits (vector engine) ----
            xsq_t, x_s, xsq_s = [], [], []
            for c in range(2):
                if MODE == "split":
                    x_s.append(hi_lo(_fp(xt_t[c]), f"x{c}"))
                xs = spool.tile([128, NLOC], MM, tag=f"xsq{c}")
                nc.scalar.square(xs[:], _fp(xt_t[c]))
                xsq_t.append(xs)
                if MODE == "split":
                    xsq_s.append(hi_lo(_fp(xs[:]), f"xsq{c}", nc.vector))

            # ---- T = G^T X^T  [KQ, NLOC] in 4 partition tiles; S = (T-g)^2 ----
            s_t, s_s = [], []
            if MODE == "split":
                # All 4 tile accumulation groups open at once (4 PSUM banks);
                # terms emitted in operand-readiness order: everything that
                # needs only the hi cast of a chunk before anything needing
                # its lo cast, chunk 0 before chunk 1.
                tts = [ttpool.tile([128, NLOC], FP, tag=f"tt{t}", name=f"tt{t}")
                       for t in range(4)]
                for ci, c in enumerate(range(2)):
                    for cls in range(3):      # 0: ghi.xhi, 1: glo.xhi, 2: ghi.xlo
                        for t in range(4):
                            ghi = gs_hi[c][:, t * 128:(t + 1) * 128]
                            glo = gs_lo[c][:, t * 128:(t + 1) * 128]
                            lh, rh = [(ghi, x_s[c][0][:]), (glo, x_s[c][0][:]),
                                      (ghi, x_s[c][1][:])][cls]
                            nc.tensor.matmul(tts[t][:], lh, rh,
                                             start=(ci == 0 and cls == 0),
                                             stop=(ci == 1 and cls == 2))
                for t in range(4):
                    s = spool.tile([128, NLOC], MM, tag=f"s{t}")
                    nc.scalar.activation(
                        s[:], tts[t][:], mybir.ActivationFunctionType.Square,
                        bias=_fp(cst_t[:, CST_GNEG + t:CST_GNEG + t + 1]),
                        scale=1.0,
                    )
                    s_t.append(s)
                    s_s.append(hi_lo(_fp(s[:]), f"s{t}", nc.vector))
            else:
                for t in range(4):
                    tt = ttpool.tile([128, NLOC], FP, tag="tt")
                    for c in range(2):
                        nc.tensor.matmul(tt[:], gs_t[c][:, t * 128:(t + 1) * 128],
                                         xt_t[c], start=(c == 0), stop=(c == 1))
                    s = spool.tile([128, NLOC], MM, tag=f"s{t}")
                    nc.scalar.activation(
                        s[:], tt[:], mybir.ActivationFunctionType.Square,
                        bias=_fp(cst_t[:, CST_GNEG + t:CST_GNEG + t + 1]),
                        scale=1.0,
                    )
                    s_t.append(s)

            # ---- single PSUM accumulation:  R = -0.5*P + U + 0.5*corr ----
            # split mode: 64-row psum; the hi-stationary terms land in rows
            # 0:32 and the lo-stationary terms in rows 32:64 (64-wide packed
            # stationaries make each 512-col moving pass do double duty);
            # the rs step sums the halves.
            rs = spool.tile([K, NLOC], FP, tag="rs")
            if MODE == "split":
                r_ps = rpool.tile([K, NLOC], FP, tag="r")
                racc = []  # (lhsT, rhs)
                for c in range(2):
                    racc += [(w2p[c][:, 0:K], x_s[c][0][:]),
                             (w2p[c][:, 0:K], x_s[c][1][:]),
                             (w2p[c][:, K:64], x_s[c][0][:]),
                             (w1p[c][:, 0:K], xsq_s[c][0][:]),
                             (w1p[c][:, 0:K], xsq_s[c][1][:]),
                             (w1p[c][:, K:64], xsq_s[c][0][:])]
                for t in range(4):
                    # ones (0.5) is exact in bf16 -> 2-term split suffices
                    racc += [(ones_t[t], s_s[t][0][:]),
                             (ones_t[t], s_s[t][1][:])]
                for i, (lhsT, rhs) in enumerate(racc):
                    nc.tensor.matmul(r_ps[:], lhsT, rhs,
                                     start=(i == 0), stop=(i == len(racc) - 1))
                nc.vector.tensor_scalar(
                    rs[:], r_ps[:], _fp(cst_t[0:K, CST_CK:CST_CK + 1]), None,
                    op0=mybir.AluOpType.add,
                )
            else:
                r_ps = rpool.tile([K, NLOC], FP, tag="r")
                racc = []
                for c in range(2):
                    racc += [(w12_t[c][:, K:64], xt_t[c]),
                             (w12_t[c][:, 0:K], xsq_t[c][:])]
                for t in range(4):
                    racc.append((ones_t[t], s_t[t][:]))
                for i, (lhsT, rhs) in enumerate(racc):
                    nc.tensor.matmul(r_ps[:], lhsT, rhs,
                                     start=(i == 0), stop=(i == len(racc) - 1))
                nc.vector.tensor_scalar(
                    rs[:], r_ps[:], _fp(cst_t[0:K, CST_CK:CST_CK + 1]), None,
                    op0=mybir.AluOpType.add,
                )

            # ---- transpose all 4 n-tiles into ONE psum bank [128, 4*K] ----
            tp = tpool.tile([128, 4 * K], FP, tag="tp")
            ident = _fp(cst_t[0:K, CST_ID:CST_ID + K])
            for j in range(4):
                nc.tensor.transpose(
                    tp[:, j * K:(j + 1) * K], rs[:, j * 128:(j + 1) * 128], ident)
            tp3 = tp[:].rearrange("p (j k) -> p j k", k=K)    # [128, 4, K]

            # ---- batched max/exp/sum of the logsumexp; pack one out tile ----
            outt = spool.tile([128, 4 * OUT_W], FP, tag="outt")
            o3 = outt[:].rearrange("p (j k) -> p j k", k=OUT_W)
            negm = o3[:, :, K]                                # [128, 4]
            nc.vector.tensor_reduce(
                o3[:, :, K:K + 1], tp3, axis=mybir.AxisListType.X,
                op=mybir.AluOpType.max, negate=True,
            )
            sh3 = o3[:, :, 0:K]
            nc.vector.tensor_tensor(
                sh3, tp3, negm.broadcast_to([128, 4, K]),
                op=mybir.AluOpType.add,                       # t - max
            )
            e = spool.tile([128, 4 * K], FP, tag="e")
            nc.scalar.activation(
                e[:].rearrange("p (j k) -> p j k", k=K), sh3,
                mybir.ActivationFunctionType.Exp)
            nc.vector.tensor_reduce(
                o3[:, :, K + 1:K + 2], e[:].rearrange("p (j k) -> p j k", k=K),
                axis=mybir.AxisListType.X, op=mybir.AluOpType.add,
            )

            nc.sync.dma_start(
                out.rearrange("(j p) k -> p j k", p=128), o3)

    nc.finalize()
    return nc


_PROGRAM_CACHE = {}


def _get_program():
    if MODE not in _PROGRAM_CACHE:
        _PROGRAM_CACHE[MODE] = _build_program()
    return _PROGRAM_CACHE[MODE]


def _bf_split(A):
    """bf16 (hi, lo) split of a float64 array."""
    hi = A.astype(ml_dtypes.bfloat16)
    lo = (A - hi.astype(np.float64)).astype(ml_dtypes.bfloat16)
    return hi, lo


def _host_prep(X, log_pi, mu, Lambda, log_psi):
    """Tiny O(K*D*Q^2) parameter prep in float64 on host."""
    X = np.asarray(X, np.float64)
    log_pi = np.asarray(log_pi, np.float64)
    mu = np.asarray(mu, np.float64)
    Lam = np.asarray(Lambda, np.float64)
    log_psi = np.asarray(log_psi, np.float64)

    a = np.exp(log_psi) + 1e-6 + 1e-5                     # [K, D]
    inv_a = 1.0 / a
    AL = Lam * inv_a[:, :, None]                          # [K, D, Q]
    B = np.eye(Q)[None] + np.einsum('kdq,kde->kqe', Lam, AL)
    R = np.linalg.cholesky(B)                             # [K, Q, Q]
    logdet = 2.0 * np.sum(np.log(np.diagonal(R, axis1=1, axis2=2)), axis=1) \
        + np.sum(np.log(a), axis=1)                       # [K]
    G = np.linalg.solve(R, AL.transpose(0, 2, 1)).transpose(0, 2, 1)  # [K, D, Q]
    g = np.einsum('kdq,kd->kq', G, mu)                    # [K, Q]
    Ck = log_pi - 0.5 * (D * LOG2PI + logdet + np.sum(mu * mu * inv_a, axis=1))

    f = np.float32
    gsm = G.transpose(1, 0, 2).reshape(D, KQ)             # G as [D, k*Q+q]
    w12 = np.concatenate([-0.5 * inv_a.T, (inv_a * mu).T], axis=1)  # [D, 64]

    cstm = np.zeros((128, CST_W), f)
    # gneg col t, partition p  <-  -g_flat[t*128 + p]  (kq index = k*Q + q)
    cstm[:, CST_GNEG:CST_GNEG + 4] = (-g).reshape(4, 128).T
    onesm = np.zeros((128, 128), f)
    for t in range(4):
        for p in range(128):
            onesm[p, 32 * t + (t * 128 + p) // Q] = 0.5
    cstm[:, CST_ONES:CST_ONES + 128] = onesm
    cstm[0:K, CST_ID:CST_ID + K] = np.eye(K, dtype=f)
    cstm[0:K, CST_CK] = Ck.astype(f)
    xt_full = np.ascontiguousarray(X.T.astype(f))         # [D, N]

    if MODE == "split":
        parb = np.zeros((D, INB_W), ml_dtypes.bfloat16)
        gh, gl = _bf_split(gsm)
        wh, wl = _bf_split(w12)
        parb[:, INB_GSHI:INB_GSHI + KQ] = gh
        parb[:, INB_GSLO:INB_GSLO + KQ] = gl
        # w2pack = [W2hi|W2lo], w1pack = [W1hi|W1lo]  (w12 = [W1 | W2])
        parb[:, INB_W2P:INB_W2P + K] = wh[:, K:64]
        parb[:, INB_W2P + K:INB_W2P + 64] = wl[:, K:64]
        parb[:, INB_W1P:INB_W1P + K] = wh[:, 0:K]
        parb[:, INB_W1P + K:INB_W1P + 64] = wl[:, 0:K]
        parb[0:128, INB_ONES:INB_ONES + 128] = onesm.astype(ml_dtypes.bfloat16)
        par = cstm                                        # [128, CST_W]
        return xt_full, par, parb
    else:
        par = np.zeros((D, IN_W - NLOC), f)               # [gs | w12 | cst]
        par[:, 0:KQ] = gsm
        par[:, KQ:KQ + 64] = w12
        par[0:128, KQ + 64:] = cstm
        return xt_full, par, None


def make_in_maps(X, log_pi, mu, Lambda, log_psi):
    xt_full, par, parb = _host_prep(X, log_pi, mu, Lambda, log_psi)
    in_maps = []
    for c in range(N_CORES):
        buf = np.zeros((D, IN_W), np.float32)
        buf[:, 0:NLOC] = xt_full[:, c * NLOC:(c + 1) * NLOC]
        if MODE == "split":
            buf[0:128, IN_CST:IN_CST + CST_W] = par
            # bf16 param block packed into the fp32 buffer (2 per slot)
            bu16 = buf.view(np.uint16)
            bu16[:, 2 * IN_BF:2 * IN_BF + INB_W] = parb.view(np.uint16)
        else:
            buf[:, NLOC:] = par
        in_maps.append({"inp": buf})
    return in_maps


def finish_outputs(results):
    """Gather per-core outputs; final scalar normalization in float64."""
    raw = np.concatenate([r["out"] for r in results], axis=0)  # [N, K+2]
    shifted = raw[:, 0:K].astype(np.float64)
    negm = raw[:, K].astype(np.float64)
    ssum = raw[:, K + 1].astype(np.float64)
    lse = np.log(ssum)                                    # [N]
    resp = (shifted - lse[:, None]).astype(np.float32)    # log_resp_norm [N, K]
    ll = (lse - negm).astype(np.float32)                  # log_likelihood [N]
    return resp, ll


def kernel(X, log_pi, mu, Lambda, log_psi):
    nc = _get_program()
    in_maps = make_in_maps(X, log_pi, mu, Lambda, log_psi)
    res = run_bass_kernel_spmd(nc, in_maps, core_ids=list(range(N_CORES)))

    return finish_outputs(res.results)


if __name__ == "__main__":
    rng = np.random.default_rng(0)
    inputs = {
        "X": rng.standard_normal((N, D)).astype(np.float32),
        "log_pi": np.full((K,), -np.log(K), np.float32),
        "mu": (0.1 * rng.standard_normal((K, D))).astype(np.float32),
        "Lambda": (0.1 * rng.standard_normal((K, D, Q))).astype(np.float32),
        "log_psi": (np.log(0.01) + 0.1 * rng.standard_normal((K, D))).astype(np.float32),
    }
    resp, ll = kernel(**inputs)
    print("resp", resp.shape, resp.dtype, "ll", ll.shape, ll.dtype)


# revision 45
# speedup vs baseline: 1.3508x; 1.0008x over previous
"""MFA E-step kernel for Trainium2 (8 NeuronCores, data-parallel over N).

Math: the reference builds C_k = Lambda_k Lambda_k^T + diag(psi_k) in [K,D,D],
Cholesky-factors it and does a triangular solve per component (~17 GFLOP).
Since C_k is diagonal-plus-rank-Q we use the Woodbury identity and the matrix
determinant lemma instead:

  C^-1 = A^-1 - A^-1 L B^-1 L^T A^-1,  B = I_Q + L^T A^-1 L,  A = diag(a)
  log|C| = log|B| + sum log a

With B = R R^T (Cholesky, [Q,Q]=16x16 - tiny, done on host) and
G = A^-1 L R^-T  [D,Q], the per-sample work reduces to

  log_resp(k, x) = -0.5 * sum_d x^2 inv_a + (inv_a*mu).x + 0.5*||G^T x - g||^2 + C_k

i.e. everything n-dependent is matmuls against X with contraction over D,
plus an elementwise square. All of that accumulates into ONE PSUM tile
[K=32, n] per core on the tensor engine. The non-matmul device steps are the
squares (scalar engine; (t-g)^2 in one instruction via per-partition bias),
a PE transpose into one PSUM bank, and the max/exp/sum of the logsumexp.
The final scalar normalization (log of the [N]-vector of exp-sums and the
broadcast subtract) happens on host during the unshard/gather step, in
float64.

Matmul precision modes (MFA_MODE):
  split (default): bf16 hi/lo 3-term products, A.B ~= Ahi.Bhi+Ahi.Blo+Alo.Bhi
        at 1 cyc/row on the PE (~2x faster than fp32's 4 cyc/row) with
        ~2^-17 per-product error - comfortably inside the fp32 envelope.
  fp32: exact fp32 (walrus lowers to 2 half-speed passes each).
  f32r: single-pass TF32-like (fast but ~2.4e-4 rel error).

Sharding: X is split along N across the 8 cores (512 rows each); the small
component parameters are replicated. No collectives needed.

I/O is packed into few large DMAs per core - the Tile runtime's
end-of-kernel drain scales with DMA queue traffic.
"""

import os

import ml_dtypes
import numpy as np

import concourse.mybir as mybir
import concourse.tile as tile
from concourse import bacc
from concourse.bass_utils import run_bass_kernel_spmd

K, D, Q, N = 32, 256, 16, 4096
N_CORES = 8
NLOC = N // N_CORES          # 512 rows of X per core
KQ = K * Q                   # 512
LOG2PI = float(np.log(2.0 * np.pi))
FP = mybir.dt.float32
BF = mybir.dt.bfloat16

MODE = os.environ.get("MFA_MODE", "split")
assert MODE in ("split", "fp32", "f32r")
MM = mybir.dt.float32r if MODE == "f32r" else mybir.dt.float32
WARMUP_MM = int(os.environ.get("MFA_WARMUP", "2"))

# fp32/f32r packed input: [xt | gs | w12 | cst];  split mode: [xt | cst] fp32
IN_XT = 0
CST_GNEG = 0                  # 4 cols:   -g per kq-tile
CST_ONES = 4                  # 128 cols: 4 blocks of [128, 32] 0.5-valued
CST_ID = CST_ONES + 128       # 32 cols:  identity (rows 0:32)
CST_CK = CST_ID + 32          # 1 col:    per-component constant (rows 0:32)
CST_W = CST_CK + 1            # 165
if MODE == "split":
    IN_CST = NLOC
    # bf16 param block [gs_hi | gs_lo | w2pack | w1pack | ones] is packed
    # INSIDE the fp32 input tensor (2 bf16 per fp32 slot, bitcast on SBUF)
    # so each chunk is a single DMA - the Tile end-of-kernel drain scales
    # with DMA count.  w2pack = [W2hi|W2lo], w1pack = [W1hi|W1lo].
    INB_GSHI = 0
    INB_GSLO = KQ
    INB_W2P = 2 * KQ
    INB_W1P = 2 * KQ + 64
    INB_ONES = 2 * KQ + 128
    INB_W = INB_ONES + 128            # 1280 bf16 = 640 fp32 cols
    IN_BF = IN_CST + CST_W            # fp32 col where the bf16 block starts
    IN_W = IN_BF + INB_W // 2
else:
    IN_GS = NLOC
    IN_W12 = IN_GS + KQ
    IN_CST = IN_W12 + 64
    IN_W = IN_CST + CST_W

OUT_W = K + 2                 # [shifted(32) | -max | sum_exp]


def _fp(ap):
    """View an MM-dtype AP as plain float32 for non-matmul consumers."""
    return ap.bitcast(FP) if MODE == "f32r" else ap


def _build_program():
    nc = bacc.Bacc("TRN2", target_bir_lowering=False)

    inp = nc.dram_tensor("inp", [D, IN_W], MM, kind="ExternalInput")
    # output stays in SBUF-tile layout [128, j*OUT_W + k] so the store DMA is
    # fully contiguous (a scattered (j p) k -> p j k pattern measured ~2.6us
    # for 68KB; contiguous is ~0.4us) - rows are reordered on host
    out = nc.dram_tensor("out", [128, 4 * OUT_W], FP, kind="ExternalOutput")

    with tile.TileContext(nc) as tc:
        with (
            tc.tile_pool(name="data", bufs=1) as dpool,
            tc.tile_pool(name="sq", bufs=1) as spool,
            tc.tile_pool(name="small", bufs=2) as vpool,
            tc.tile_pool(name="warm", bufs=1) as wpool,
            tc.tile_pool(name="ttps", bufs=1 if MODE == "split" else 2,
                         space="PSUM") as ttpool,
            tc.tile_pool(name="rps", bufs=1, space="PSUM") as rpool,
            tc.tile_pool(name="tps", bufs=1, space="PSUM") as tpool,
            tc.tile_pool(name="wps", bufs=1, space="PSUM") as wpspool,
        ):
            # ---- PE warm-up: keep the tensor engine busy through the DMA
            # phase so HAM un-throttles (1.2 -> 2.4 GHz) before real work.
            if WARMUP_MM:
                wsrc = wpool.tile([128, NLOC], FP, tag="wsrc")
                nc.vector.memset(wsrc[:], 0.0)
                wps = wpspool.tile([128, NLOC], FP, tag="wps")
                for i in range(WARMUP_MM):
                    nc.tensor.matmul(wps[:], wsrc[:, 0:128], wsrc[:],
                                     start=(i == 0), stop=(i == WARMUP_MM - 1))

            # ---- loads: one packed DMA per chunk per dram tensor, split
            # across the two HWDGE issuing engines (Sync / Scalar) ----
            # one packed DMA per chunk, one per HWDGE issuing engine
            in_t = []
            for c in range(2):
                t = dpool.tile([128, IN_W], MM, tag=f"in{c}")
                eng = nc.sync if c == 0 else nc.scalar
                eng.dma_start(t[:], inp[c * 128:(c + 1) * 128, :])
                in_t.append(t)
            xt_t = [in_t[c][:, IN_XT:IN_XT + NLOC] for c in range(2)]
            cst_t = in_t[0][:, IN_CST:IN_CST + CST_W]
            if MODE == "split":
                inb_t = [in_t[c][:, IN_BF:IN_BF + INB_W // 2].bitcast(BF)
                         for c in range(2)]
                gs_hi = [inb_t[c][:, INB_GSHI:INB_GSHI + KQ] for c in range(2)]
                gs_lo = [inb_t[c][:, INB_GSLO:INB_GSLO + KQ] for c in range(2)]
                w2p = [inb_t[c][:, INB_W2P:INB_W2P + 64] for c in range(2)]
                w1p = [inb_t[c][:, INB_W1P:INB_W1P + 64] for c in range(2)]
                ones_t = [inb_t[0][:, INB_ONES + 32 * t:INB_ONES + 32 * (t + 1)]
                          for t in range(4)]
            else:
                gs_t = [in_t[c][:, IN_GS:IN_GS + KQ] for c in range(2)]
                w12_t = [in_t[c][:, IN_W12:IN_W12 + 64] for c in range(2)]
                ones_t = [cst_t[:, CST_ONES + 32 * t:CST_ONES + 32 * (t + 1)]
                          for t in range(4)]

            def hi_lo(src_fp, tag, hi_eng=None):
                """bf16 split of a [128, NLOC] fp32 AP: hi = bf16(x) on
                hi_eng (scalar/gpsimd - spreads cast load off the critical
                engine), lo = bf16(x - hi) on the vector engine."""
                hi = spool.tile([128, NLOC], BF, tag=f"{tag}hi")
                if hi_eng is None:
                    nc.scalar.copy(hi[:], src_fp)
                else:
                    hi_eng.tensor_copy(hi[:], src_fp)
                lo = spool.tile([128, NLOC], BF, tag=f"{tag}lo")
                nc.vector.tensor_tensor(lo[:], src_fp, hi[:],
                                        op=mybir.AluOpType.subtract)
                return hi, lo

            # ---- x^2 (scalar engine), bf16 splits (vector engine) ----
            xsq_t, x_s, xsq_s = [], [], []
            for c in range(2):
                if MODE == "split":
                    x_s.append(hi_lo(_fp(xt_t[c]), f"x{c}"))
                xs = spool.tile([128, NLOC], MM, tag=f"xsq{c}")
                nc.scalar.square(xs[:], _fp(xt_t[c]))
                xsq_t.append(xs)
                if MODE == "split":
                    xsq_s.append(hi_lo(_fp(xs[:]), f"xsq{c}", nc.vector))

            # ---- T = G^T X^T  [KQ, NLOC] in 4 partition tiles; S = (T-g)^2 ----
            s_t, s_s = [], []
            if MODE == "split":
                # All 4 tile accumulation groups open at once (4 PSUM banks);
                # terms emitted in operand-readiness order: everything that
                # needs only the hi cast of a chunk before anything needing
                # its lo cast, chunk 0 before chunk 1.
                tts = [ttpool.tile([128, NLOC], FP, tag=f"tt{t}", name=f"tt{t}")
                       for t in range(4)]
                for ci, c in enumerate(range(2)):
                    for cls in range(3):      # 0: ghi.xhi, 1: glo.xhi, 2: ghi.xlo
                        for t in range(4):
                            ghi = gs_hi[c][:, t * 128:(t + 1) * 128]
                            glo = gs_lo[c][:, t * 128:(t + 1) * 128]
                            lh, rh = [(ghi, x_s[c][0][:]), (glo, x_s[c][0][:]),
                                      (ghi, x_s[c][1][:])][cls]
                            nc.tensor.matmul(tts[t][:], lh, rh,
                                             start=(ci == 0 and cls == 0),
                                             stop=(ci == 1 and cls == 2))
                for t in range(4):
                    s = spool.tile([128, NLOC], MM, tag=f"s{t}")
                    nc.scalar.activation(
                        s[:], tts[t][:], mybir.ActivationFunctionType.Square,
                        bias=_fp(cst_t[:, CST_GNEG + t:CST_GNEG + t + 1]),
                        scale=1.0,
                    )
                    s_t.append(s)
                    s_s.append(hi_lo(_fp(s[:]), f"s{t}", nc.vector))
            else:
                for t in range(4):
                    tt = ttpool.tile([128, NLOC], FP, tag="tt")
                    for c in range(2):
                        nc.tensor.matmul(tt[:], gs_t[c][:, t * 128:(t + 1) * 128],
                                         xt_t[c], start=(c == 0), stop=(c == 1))
                    s = spool.tile([128, NLOC], MM, tag=f"s{t}")
                    nc.scalar.activation(
                        s[:], tt[:], mybir.ActivationFunctionType.Square,
                        bias=_fp(cst_t[:, CST_GNEG + t:CST_GNEG + t + 1]),
                        scale=1.0,
                    )
                    s_t.append(s)

            # ---- single PSUM accumulation:  R = -0.5*P + U + 0.5*corr ----
            # split mode: 64-row psum; the hi-stationary terms land in rows
            # 0:32 and the lo-stationary terms in rows 32:64 (64-wide packed
            # stationaries make each 512-col moving pass do double duty);
            # the rs step sums the halves.
            rs = spool.tile([K, NLOC], FP, tag="rs")
            if MODE == "split":
                r_ps = rpool.tile([K, NLOC], FP, tag="r")
                racc = []  # (lhsT, rhs)
                for c in range(2):
                    racc += [(w2p[c][:, 0:K], x_s[c][0][:]),
                             (w2p[c][:, 0:K], x_s[c][1][:]),
                             (w2p[c][:, K:64], x_s[c][0][:]),
                             (w1p[c][:, 0:K], xsq_s[c][0][:]),
                             (w1p[c][:, 0:K], xsq_s[c][1][:]),
                             (w1p[c][:, K:64], xsq_s[c][0][:])]
                for t in range(4):
                    # ones (0.5) is exact in bf16 -> 2-term split suffices
                    racc += [(ones_t[t], s_s[t][0][:]),
                             (ones_t[t], s_s[t][1][:])]
                for i, (lhsT, rhs) in enumerate(racc):
                    nc.tensor.matmul(r_ps[:], lhsT, rhs,
                                     start=(i == 0), stop=(i == len(racc) - 1))
                nc.vector.tensor_scalar(
                    rs[:], r_ps[:], _fp(cst_t[0:K, CST_CK:CST_CK + 1]), None,
                    op0=mybir.AluOpType.add,
                )
            else:
                r_ps = rpool.tile([K, NLOC], FP, tag="r")
                racc = []
                for c in range(2):
                    racc += [(w12_t[c][:, K:64], xt_t[c]),
                             (w12_t[c][:, 0:K], xsq_t[c][:])]
                for t in range(4):
                    racc.append((ones_t[t], s_t[t][:]))
                for i, (lhsT, rhs) in enumerate(racc):
                    nc.tensor.matmul(r_ps[:], lhsT, rhs,
                                     start=(i == 0), stop=(i == len(racc) - 1))
                nc.vector.tensor_scalar(
                    rs[:], r_ps[:], _fp(cst_t[0:K, CST_CK:CST_CK + 1]), None,
                    op0=mybir.AluOpType.add,
                )

            # ---- transpose all 4 n-tiles into ONE psum bank [128, 4*K] ----
            tp = tpool.tile([128, 4 * K], FP, tag="tp")
            ident = _fp(cst_t[0:K, CST_ID:CST_ID + K])
            for j in range(4):
                nc.tensor.transpose(
                    tp[:, j * K:(j + 1) * K], rs[:, j * 128:(j + 1) * 128], ident)
            tp3 = tp[:].rearrange("p (j k) -> p j k", k=K)    # [128, 4, K]

            # ---- batched max/exp/sum of the logsumexp; pack one out tile ----
            outt = spool.tile([128, 4 * OUT_W], FP, tag="outt")
            o3 = outt[:].rearrange("p (j k) -> p j k", k=OUT_W)
            negm = o3[:, :, K]                                # [128, 4]
            nc.vector.tensor_reduce(
                o3[:, :, K:K + 1], tp3, axis=mybir.AxisListType.X,
                op=mybir.AluOpType.max, negate=True,
            )
            sh3 = o3[:, :, 0:K]
            nc.vector.tensor_tensor(
                sh3, tp3, negm.broadcast_to([128, 4, K]),
                op=mybir.AluOpType.add,                       # t - max
            )
            e = spool.tile([128, 4 * K], FP, tag="e")
            nc.scalar.activation(
                e[:].rearrange("p (j k) -> p j k", k=K), sh3,
                mybir.ActivationFunctionType.Exp)
            nc.vector.tensor_reduce(
                o3[:, :, K + 1:K + 2], e[:].rearrange("p (j k) -> p j k", k=K),
                axis=mybir.AxisListType.X, op=mybir.AluOpType.add,
            )

            nc.sync.dma_start(out[:, :], outt[:])

    nc.finalize()
    return nc


_PROGRAM_CACHE = {}


def _get_program():
    if MODE not in _PROGRAM_CACHE:
        _PROGRAM_CACHE[MODE] = _build_program()
    return _PROGRAM_CACHE[MODE]


def _bf_split(A):
    """bf16 (hi, lo) split of a float64 array."""
    hi = A.astype(ml_dtypes.bfloat16)
    lo = (A - hi.astype(np.float64)).astype(ml_dtypes.bfloat16)
    return hi, lo


def _host_prep(X, log_pi, mu, Lambda, log_psi):
    """Tiny O(K*D*Q^2) parameter prep in float64 on host."""
    X = np.asarray(X, np.float64)
    log_pi = np.asarray(log_pi, np.float64)
    mu = np.asarray(mu, np.float64)
    Lam = np.asarray(Lambda, np.float64)
    log_psi = np.asarray(log_psi, np.float64)

    a = np.exp(log_psi) + 1e-6 + 1e-5                     # [K, D]
    inv_a = 1.0 / a
    AL = Lam * inv_a[:, :, None]                          # [K, D, Q]
    B = np.eye(Q)[None] + np.einsum('kdq,kde->kqe', Lam, AL)
    R = np.linalg.cholesky(B)                             # [K, Q, Q]
    logdet = 2.0 * np.sum(np.log(np.diagonal(R, axis1=1, axis2=2)), axis=1) \
        + np.sum(np.log(a), axis=1)                       # [K]
    G = np.linalg.solve(R, AL.transpose(0, 2, 1)).transpose(0, 2, 1)  # [K, D, Q]
    g = np.einsum('kdq,kd->kq', G, mu)                    # [K, Q]
    Ck = log_pi - 0.5 * (D * LOG2PI + logdet + np.sum(mu * mu * inv_a, axis=1))

    f = np.float32
    gsm = G.transpose(1, 0, 2).reshape(D, KQ)             # G as [D, k*Q+q]
    w12 = np.concatenate([-0.5 * inv_a.T, (inv_a * mu).T], axis=1)  # [D, 64]

    cstm = np.zeros((128, CST_W), f)
    # gneg col t, partition p  <-  -g_flat[t*128 + p]  (kq index = k*Q + q)
    cstm[:, CST_GNEG:CST_GNEG + 4] = (-g).reshape(4, 128).T
    onesm = np.zeros((128, 128), f)
    for t in range(4):
        for p in range(128):
            onesm[p, 32 * t + (t * 128 + p) // Q] = 0.5
    cstm[:, CST_ONES:CST_ONES + 128] = onesm
    cstm[0:K, CST_ID:CST_ID + K] = np.eye(K, dtype=f)
    cstm[0:K, CST_CK] = Ck.astype(f)
    xt_full = np.ascontiguousarray(X.T.astype(f))         # [D, N]

    if MODE == "split":
        parb = np.zeros((D, INB_W), ml_dtypes.bfloat16)
        gh, gl = _bf_split(gsm)
        wh, wl = _bf_split(w12)
        parb[:, INB_GSHI:INB_GSHI + KQ] = gh
        parb[:, INB_GSLO:INB_GSLO + KQ] = gl
        # w2pack = [W2hi|W2lo], w1pack = [W1hi|W1lo]  (w12 = [W1 | W2])
        parb[:, INB_W2P:INB_W2P + K] = wh[:, K:64]
        parb[:, INB_W2P + K:INB_W2P + 64] = wl[:, K:64]
        parb[:, INB_W1P:INB_W1P + K] = wh[:, 0:K]
        parb[:, INB_W1P + K:INB_W1P + 64] = wl[:, 0:K]
        parb[0:128, INB_ONES:INB_ONES + 128] = onesm.astype(ml_dtypes.bfloat16)
        par = cstm                                        # [128, CST_W]
        return xt_full, par, parb
    else:
        par = np.zeros((D, IN_W - NLOC), f)               # [gs | w12 | cst]
        par[:, 0:KQ] = gsm
        par[:, KQ:KQ + 64] = w12
        par[0:128, KQ + 64:] = cstm
        return xt_full, par, None


def make_in_maps(X, log_pi, mu, Lambda, log_psi):
    xt_full, par, parb = _host_prep(X, log_pi, mu, Lambda, log_psi)
    in_maps = []
    for c in range(N_CORES):
        buf = np.zeros((D, IN_W), np.float32)
        buf[:, 0:NLOC] = xt_full[:, c * NLOC:(c + 1) * NLOC]
        if MODE == "split":
            buf[0:128, IN_CST:IN_CST + CST_W] = par
            # bf16 param block packed into the fp32 buffer (2 per slot)
            bu16 = buf.view(np.uint16)
            bu16[:, 2 * IN_BF:2 * IN_BF + INB_W] = parb.view(np.uint16)
        else:
            buf[:, NLOC:] = par
        in_maps.append({"inp": buf})
    return in_maps


def finish_outputs(results):
    """Gather per-core outputs; final scalar normalization in float64."""
    raw = np.concatenate(
        [r["out"].reshape(128, 4, OUT_W).transpose(1, 0, 2).reshape(NLOC, OUT_W)
         for r in results], axis=0)                       # [N, K+2]
    shifted = raw[:, 0:K].astype(np.float64)
    negm = raw[:, K].astype(np.float64)
    ssum = raw[:, K + 1].astype(np.float64)
    lse = np.log(ssum)                                    # [N]
    resp = (shifted - lse[:, None]).astype(np.float32)    # log_resp_norm [N, K]
    ll = (lse - negm).astype(np.float32)                  # log_likelihood [N]
    return resp, ll


def kernel(X, log_pi, mu, Lambda, log_psi):
    nc = _get_program()
    in_maps = make_in_maps(X, log_pi, mu, Lambda, log_psi)
    res = run_bass_kernel_spmd(nc, in_maps, core_ids=list(range(N_CORES)))

    return finish_outputs(res.results)


if __name__ == "__main__":
    rng = np.random.default_rng(0)
    inputs = {
        "X": rng.standard_normal((N, D)).astype(np.float32),
        "log_pi": np.full((K,), -np.log(K), np.float32),
        "mu": (0.1 * rng.standard_normal((K, D))).astype(np.float32),
        "Lambda": (0.1 * rng.standard_normal((K, D, Q))).astype(np.float32),
        "log_psi": (np.log(0.01) + 0.1 * rng.standard_normal((K, D))).astype(np.float32),
    }
    resp, ll = kernel(**inputs)
    print("resp", resp.shape, resp.dtype, "ll", ll.shape, ll.dtype)


# revision 48
# speedup vs baseline: 1.3718x; 1.0156x over previous
"""MFA E-step kernel for Trainium2 (8 NeuronCores, data-parallel over N).

Math: the reference builds C_k = Lambda_k Lambda_k^T + diag(psi_k) in [K,D,D],
Cholesky-factors it and does a triangular solve per component (~17 GFLOP).
Since C_k is diagonal-plus-rank-Q we use the Woodbury identity and the matrix
determinant lemma instead:

  C^-1 = A^-1 - A^-1 L B^-1 L^T A^-1,  B = I_Q + L^T A^-1 L,  A = diag(a)
  log|C| = log|B| + sum log a

With B = R R^T (Cholesky, [Q,Q]=16x16 - tiny, done on host) and
G = A^-1 L R^-T  [D,Q], the per-sample work reduces to

  log_resp(k, x) = -0.5 * sum_d x^2 inv_a + (inv_a*mu).x + 0.5*||G^T x - g||^2 + C_k

i.e. everything n-dependent is matmuls against X with contraction over D,
plus an elementwise square. All of that accumulates into ONE PSUM tile
[K=32, n] per core on the tensor engine. The non-matmul device steps are the
squares (scalar engine; (t-g)^2 in one instruction via per-partition bias),
a PE transpose into one PSUM bank, and the max/exp/sum of the logsumexp.
The final scalar normalization (log of the [N]-vector of exp-sums and the
broadcast subtract) happens on host during the unshard/gather step, in
float64.

Matmul precision modes (MFA_MODE):
  split (default): bf16 hi/lo 3-term products, A.B ~= Ahi.Bhi+Ahi.Blo+Alo.Bhi
        at 1 cyc/row on the PE (~2x faster than fp32's 4 cyc/row) with
        ~2^-17 per-product error - comfortably inside the fp32 envelope.
  fp32: exact fp32 (walrus lowers to 2 half-speed passes each).
  f32r: single-pass TF32-like (fast but ~2.4e-4 rel error).

Sharding: X is split along N across the 8 cores (512 rows each); the small
component parameters are replicated. No collectives needed.

I/O is packed into few large DMAs per core - the Tile runtime's
end-of-kernel drain scales with DMA queue traffic.
"""

import os

import ml_dtypes
import numpy as np

import concourse.mybir as mybir
import concourse.tile as tile
from concourse import bacc
from concourse.bass_utils import run_bass_kernel_spmd

K, D, Q, N = 32, 256, 16, 4096
N_CORES = 8
NLOC = N // N_CORES          # 512 rows of X per core
KQ = K * Q                   # 512
LOG2PI = float(np.log(2.0 * np.pi))
FP = mybir.dt.float32
BF = mybir.dt.bfloat16

MODE = os.environ.get("MFA_MODE", "split")
assert MODE in ("split", "fp32", "f32r")
MM = mybir.dt.float32r if MODE == "f32r" else mybir.dt.float32
WARMUP_MM = int(os.environ.get("MFA_WARMUP", "2"))

# fp32/f32r packed input: [xt | gs | w12 | cst];  split mode: [xt | cst] fp32
IN_XT = 0
CST_GNEG = 0                  # 4 cols:   -g per kq-tile
CST_ONES = 4                  # 128 cols: 4 blocks of [128, 32] 0.5-valued
CST_ID = CST_ONES + 128       # 32 cols:  identity (rows 0:32)
CST_CK = CST_ID + 32          # 1 col:    per-component constant (rows 0:32)
CST_W = CST_CK + 1            # 165
if MODE == "split":
    IN_CST = NLOC
    # bf16 param block [gs_hi | gs_lo | w2pack | w1pack | ones] is packed
    # INSIDE the fp32 input tensor (2 bf16 per fp32 slot, bitcast on SBUF)
    # so each chunk is a single DMA - the Tile end-of-kernel drain scales
    # with DMA count.  w2pack = [W2hi|W2lo], w1pack = [W1hi|W1lo].
    INB_GSHI = 0
    INB_GSLO = KQ
    INB_W2P = 2 * KQ
    INB_W1P = 2 * KQ + 64
    INB_ONES = 2 * KQ + 128
    INB_W = INB_ONES + 128            # 1280 bf16 = 640 fp32 cols
    IN_BF = IN_CST + CST_W            # fp32 col where the bf16 block starts
    IN_W = IN_BF + INB_W // 2
else:
    IN_GS = NLOC
    IN_W12 = IN_GS + KQ
    IN_CST = IN_W12 + 64
    IN_W = IN_CST + CST_W

OUT_W = K + 2                 # [shifted(32) | -max | sum_exp]


def _fp(ap):
    """View an MM-dtype AP as plain float32 for non-matmul consumers."""
    return ap.bitcast(FP) if MODE == "f32r" else ap


def _build_program():
    nc = bacc.Bacc("TRN2", target_bir_lowering=False)

    inp = nc.dram_tensor("inp", [D, IN_W], MM, kind="ExternalInput")
    # output stays in SBUF-tile layout [128, j*OUT_W + k] so the store DMA is
    # fully contiguous (a scattered (j p) k -> p j k pattern measured ~2.6us
    # for 68KB; contiguous is ~0.4us) - rows are reordered on host
    out = nc.dram_tensor("out", [128, 4 * OUT_W], FP, kind="ExternalOutput")

    with tile.TileContext(nc) as tc:
        with (
            tc.tile_pool(name="data", bufs=1) as dpool,
            tc.tile_pool(name="sq", bufs=1) as spool,
            tc.tile_pool(name="small", bufs=2) as vpool,
            tc.tile_pool(name="warm", bufs=1) as wpool,
            tc.tile_pool(name="ttps", bufs=1 if MODE == "split" else 2,
                         space="PSUM") as ttpool,
            tc.tile_pool(name="rps", bufs=1, space="PSUM") as rpool,
            tc.tile_pool(name="tps", bufs=1, space="PSUM") as tpool,
            tc.tile_pool(name="wps", bufs=1, space="PSUM") as wpspool,
        ):
            # ---- PE warm-up: keep the tensor engine busy through the DMA
            # phase so HAM un-throttles (1.2 -> 2.4 GHz) before real work.
            if WARMUP_MM:
                wsrc = wpool.tile([128, NLOC], FP, tag="wsrc")
                nc.vector.memset(wsrc[:], 0.0)
                wps = wpspool.tile([128, NLOC], FP, tag="wps")
                for i in range(WARMUP_MM):
                    nc.tensor.matmul(wps[:], wsrc[:, 0:128], wsrc[:],
                                     start=(i == 0), stop=(i == WARMUP_MM - 1))

            # ---- loads: one packed DMA per chunk per dram tensor, split
            # across the two HWDGE issuing engines (Sync / Scalar) ----
            # one packed DMA per chunk, one per HWDGE issuing engine
            in_t = []
            for c in range(2):
                t = dpool.tile([128, IN_W], MM, tag=f"in{c}")
                eng = nc.sync if c == 0 else nc.scalar
                eng.dma_start(t[:], inp[c * 128:(c + 1) * 128, :])
                in_t.append(t)
            xt_t = [in_t[c][:, IN_XT:IN_XT + NLOC] for c in range(2)]
            cst_t = in_t[0][:, IN_CST:IN_CST + CST_W]
            if MODE == "split":
                inb_t = [in_t[c][:, IN_BF:IN_BF + INB_W // 2].bitcast(BF)
                         for c in range(2)]
                gs_hi = [inb_t[c][:, INB_GSHI:INB_GSHI + KQ] for c in range(2)]
                gs_lo = [inb_t[c][:, INB_GSLO:INB_GSLO + KQ] for c in range(2)]
                w2p = [inb_t[c][:, INB_W2P:INB_W2P + 64] for c in range(2)]
                w1p = [inb_t[c][:, INB_W1P:INB_W1P + 64] for c in range(2)]
                ones_t = [inb_t[0][:, INB_ONES + 32 * t:INB_ONES + 32 * (t + 1)]
                          for t in range(4)]
            else:
                gs_t = [in_t[c][:, IN_GS:IN_GS + KQ] for c in range(2)]
                w12_t = [in_t[c][:, IN_W12:IN_W12 + 64] for c in range(2)]
                ones_t = [cst_t[:, CST_ONES + 32 * t:CST_ONES + 32 * (t + 1)]
                          for t in range(4)]

            def hi_lo(src_fp, tag, hi_eng=None):
                """bf16 split of a [128, NLOC] fp32 AP: hi = bf16(x) on
                hi_eng (scalar/gpsimd - spreads cast load off the critical
                engine), lo = bf16(x - hi) on the vector engine."""
                hi = spool.tile([128, NLOC], BF, tag=f"{tag}hi")
                if hi_eng is None:
                    nc.scalar.copy(hi[:], src_fp)
                else:
                    hi_eng.tensor_copy(hi[:], src_fp)
                lo = spool.tile([128, NLOC], BF, tag=f"{tag}lo")
                nc.vector.tensor_tensor(lo[:], src_fp, hi[:],
                                        op=mybir.AluOpType.subtract)
                return hi, lo

            # ---- x^2 (scalar engine), bf16 splits (vector engine) ----
            xsq_t, x_s, xsq_s = [], [], []
            for c in range(2):
                if MODE == "split":
                    x_s.append(hi_lo(_fp(xt_t[c]), f"x{c}"))
                xs = spool.tile([128, NLOC], MM, tag=f"xsq{c}")
                nc.scalar.square(xs[:], _fp(xt_t[c]))
                xsq_t.append(xs)
                if MODE == "split":
                    xsq_s.append(hi_lo(_fp(xs[:]), f"xsq{c}", nc.vector))

            # ---- T = G^T X^T  [KQ, NLOC] in 4 partition tiles; S = (T-g)^2 ----
            s_t, s_s = [], []
            if MODE == "split":
                # All 4 tile accumulation groups open at once (4 PSUM banks);
                # terms emitted in operand-readiness order: everything that
                # needs only the hi cast of a chunk before anything needing
                # its lo cast, chunk 0 before chunk 1.
                tts = [ttpool.tile([128, NLOC], FP, tag=f"tt{t}", name=f"tt{t}")
                       for t in range(4)]
                for ci, c in enumerate(range(2)):
                    for cls in range(3):      # 0: ghi.xhi, 1: glo.xhi, 2: ghi.xlo
                        for t in range(4):
                            ghi = gs_hi[c][:, t * 128:(t + 1) * 128]
                            glo = gs_lo[c][:, t * 128:(t + 1) * 128]
                            lh, rh = [(ghi, x_s[c][0][:]), (glo, x_s[c][0][:]),
                                      (ghi, x_s[c][1][:])][cls]
                            nc.tensor.matmul(tts[t][:], lh, rh,
                                             start=(ci == 0 and cls == 0),
                                             stop=(ci == 1 and cls == 2))
                for t in range(4):
                    s = spool.tile([128, NLOC], MM, tag=f"s{t}")
                    nc.scalar.activation(
                        s[:], tts[t][:], mybir.ActivationFunctionType.Square,
                        bias=_fp(cst_t[:, CST_GNEG + t:CST_GNEG + t + 1]),
                        scale=1.0,
                    )
                    s_t.append(s)
                    s_s.append(hi_lo(_fp(s[:]), f"s{t}", nc.vector))
            else:
                for t in range(4):
                    tt = ttpool.tile([128, NLOC], FP, tag="tt")
                    for c in range(2):
                        nc.tensor.matmul(tt[:], gs_t[c][:, t * 128:(t + 1) * 128],
                                         xt_t[c], start=(c == 0), stop=(c == 1))
                    s = spool.tile([128, NLOC], MM, tag=f"s{t}")
                    nc.scalar.activation(
                        s[:], tt[:], mybir.ActivationFunctionType.Square,
                        bias=_fp(cst_t[:, CST_GNEG + t:CST_GNEG + t + 1]),
                        scale=1.0,
                    )
                    s_t.append(s)

            # ---- single PSUM accumulation:  R = -0.5*P + U + 0.5*corr ----
            # split mode: 64-row psum; the hi-stationary terms land in rows
            # 0:32 and the lo-stationary terms in rows 32:64 (64-wide packed
            # stationaries make each 512-col moving pass do double duty);
            # the rs step sums the halves.
            rs = spool.tile([K, NLOC], FP, tag="rs")
            if MODE == "split":
                r_ps = rpool.tile([K, NLOC], FP, tag="r")
                racc = []  # (lhsT, rhs)
                for c in range(2):
                    racc += [(w2p[c][:, 0:K], x_s[c][0][:]),
                             (w2p[c][:, 0:K], x_s[c][1][:]),
                             (w2p[c][:, K:64], x_s[c][0][:]),
                             (w1p[c][:, 0:K], xsq_s[c][0][:]),
                             (w1p[c][:, 0:K], xsq_s[c][1][:]),
                             (w1p[c][:, K:64], xsq_s[c][0][:])]
                for t in range(4):
                    # ones (0.5) is exact in bf16 -> 2-term split suffices
                    racc += [(ones_t[t], s_s[t][0][:]),
                             (ones_t[t], s_s[t][1][:])]
                for i, (lhsT, rhs) in enumerate(racc):
                    nc.tensor.matmul(r_ps[:], lhsT, rhs,
                                     start=(i == 0), stop=(i == len(racc) - 1))
                nc.vector.tensor_scalar(
                    rs[:], r_ps[:], _fp(cst_t[0:K, CST_CK:CST_CK + 1]), None,
                    op0=mybir.AluOpType.add,
                )
            else:
                r_ps = rpool.tile([K, NLOC], FP, tag="r")
                racc = []
                for c in range(2):
                    racc += [(w12_t[c][:, K:64], xt_t[c]),
                             (w12_t[c][:, 0:K], xsq_t[c][:])]
                for t in range(4):
                    racc.append((ones_t[t], s_t[t][:]))
                for i, (lhsT, rhs) in enumerate(racc):
                    nc.tensor.matmul(r_ps[:], lhsT, rhs,
                                     start=(i == 0), stop=(i == len(racc) - 1))
                nc.vector.tensor_scalar(
                    rs[:], r_ps[:], _fp(cst_t[0:K, CST_CK:CST_CK + 1]), None,
                    op0=mybir.AluOpType.add,
                )

            # ---- transpose all 4 n-tiles into ONE psum bank [128, 4*K] ----
            tp = tpool.tile([128, 4 * K], FP, tag="tp")
            ident = _fp(cst_t[0:K, CST_ID:CST_ID + K])
            for j in range(4):
                nc.tensor.transpose(
                    tp[:, j * K:(j + 1) * K], rs[:, j * 128:(j + 1) * 128], ident)
            tp3 = tp[:].rearrange("p (j k) -> p j k", k=K)    # [128, 4, K]

            # ---- batched max/exp/sum of the logsumexp; pack one out tile ----
            outt = spool.tile([128, 4 * OUT_W], FP, tag="outt")
            o3 = outt[:].rearrange("p (j k) -> p j k", k=OUT_W)
            negm = o3[:, :, K]                                # [128, 4]
            nc.vector.tensor_reduce(
                o3[:, :, K:K + 1], tp3, axis=mybir.AxisListType.X,
                op=mybir.AluOpType.max, negate=True,
            )
            sh3 = o3[:, :, 0:K]
            nc.vector.tensor_tensor(
                sh3, tp3, negm.broadcast_to([128, 4, K]),
                op=mybir.AluOpType.add,                       # t - max
            )
            e = spool.tile([128, 4 * K], FP, tag="e")
            nc.scalar.activation(
                e[:].rearrange("p (j k) -> p j k", k=K), sh3,
                mybir.ActivationFunctionType.Exp)
            nc.vector.tensor_reduce(
                o3[:, :, K + 1:K + 2], e[:].rearrange("p (j k) -> p j k", k=K),
                axis=mybir.AxisListType.X, op=mybir.AluOpType.add,
            )

            nc.sync.dma_start(out[:, :], outt[:])

    nc.finalize()
    return nc


_PROGRAM_CACHE = {}


def _get_program():
    if MODE not in _PROGRAM_CACHE:
        _PROGRAM_CACHE[MODE] = _build_program()
    return _PROGRAM_CACHE[MODE]


def _bf_split(A):
    """bf16 (hi, lo) split of a float64 array."""
    hi = A.astype(ml_dtypes.bfloat16)
    lo = (A - hi.astype(np.float64)).astype(ml_dtypes.bfloat16)
    return hi, lo


def _host_prep(X, log_pi, mu, Lambda, log_psi):
    """Tiny O(K*D*Q^2) parameter prep in float64 on host."""
    X = np.asarray(X, np.float64)
    log_pi = np.asarray(log_pi, np.float64)
    mu = np.asarray(mu, np.float64)
    Lam = np.asarray(Lambda, np.float64)
    log_psi = np.asarray(log_psi, np.float64)

    a = np.exp(log_psi) + 1e-6 + 1e-5                     # [K, D]
    inv_a = 1.0 / a
    AL = Lam * inv_a[:, :, None]                          # [K, D, Q]
    B = np.eye(Q)[None] + np.einsum('kdq,kde->kqe', Lam, AL)
    R = np.linalg.cholesky(B)                             # [K, Q, Q]
    logdet = 2.0 * np.sum(np.log(np.diagonal(R, axis1=1, axis2=2)), axis=1) \
        + np.sum(np.log(a), axis=1)                       # [K]
    G = np.linalg.solve(R, AL.transpose(0, 2, 1)).transpose(0, 2, 1)  # [K, D, Q]
    g = np.einsum('kdq,kd->kq', G, mu)                    # [K, Q]
    Ck = log_pi - 0.5 * (D * LOG2PI + logdet + np.sum(mu * mu * inv_a, axis=1))

    f = np.float32
    gsm = G.transpose(1, 0, 2).reshape(D, KQ)             # G as [D, k*Q+q]
    w12 = np.concatenate([-0.5 * inv_a.T, (inv_a * mu).T], axis=1)  # [D, 64]

    cstm = np.zeros((128, CST_W), f)
    # gneg col t, partition p  <-  -g_flat[t*128 + p]  (kq index = k*Q + q)
    cstm[:, CST_GNEG:CST_GNEG + 4] = (-g).reshape(4, 128).T
    onesm = np.zeros((128, 128), f)
    for t in range(4):
        for p in range(128):
            onesm[p, 32 * t + (t * 128 + p) // Q] = 0.5
    cstm[:, CST_ONES:CST_ONES + 128] = onesm
    cstm[0:K, CST_ID:CST_ID + K] = np.eye(K, dtype=f)
    cstm[0:K, CST_CK] = Ck.astype(f)
    xt_full = np.ascontiguousarray(X.T.astype(f))         # [D, N]

    if MODE == "split":
        parb = np.zeros((D, INB_W), ml_dtypes.bfloat16)
        gh, gl = _bf_split(gsm)
        wh, wl = _bf_split(w12)
        parb[:, INB_GSHI:INB_GSHI + KQ] = gh
        parb[:, INB_GSLO:INB_GSLO + KQ] = gl
        # w2pack = [W2hi|W2lo], w1pack = [W1hi|W1lo]  (w12 = [W1 | W2])
        parb[:, INB_W2P:INB_W2P + K] = wh[:, K:64]
        parb[:, INB_W2P + K:INB_W2P + 64] = wl[:, K:64]
        parb[:, INB_W1P:INB_W1P + K] = wh[:, 0:K]
        parb[:, INB_W1P + K:INB_W1P + 64] = wl[:, 0:K]
        parb[0:128, INB_ONES:INB_ONES + 128] = onesm.astype(ml_dtypes.bfloat16)
        par = cstm                                        # [128, CST_W]
        return xt_full, par, parb
    else:
        par = np.zeros((D, IN_W - NLOC), f)               # [gs | w12 | cst]
        par[:, 0:KQ] = gsm
        par[:, KQ:KQ + 64] = w12
        par[0:128, KQ + 64:] = cstm
        return xt_full, par, None


def make_in_maps(X, log_pi, mu, Lambda, log_psi):
    xt_full, par, parb = _host_prep(X, log_pi, mu, Lambda, log_psi)
    in_maps = []
    for c in range(N_CORES):
        buf = np.zeros((D, IN_W), np.float32)
        buf[:, 0:NLOC] = xt_full[:, c * NLOC:(c + 1) * NLOC]
        if MODE == "split":
            buf[0:128, IN_CST:IN_CST + CST_W] = par
            # bf16 param block packed into the fp32 buffer (2 per slot)
            bu16 = buf.view(np.uint16)
            bu16[:, 2 * IN_BF:2 * IN_BF + INB_W] = parb.view(np.uint16)
        else:
            buf[:, NLOC:] = par
        in_maps.append({"inp": buf})
    return in_maps


def finish_outputs(results):
    """Gather per-core outputs; final scalar normalization in float64."""
    raw = np.concatenate(
        [r["out"].reshape(128, 4, OUT_W).transpose(1, 0, 2).reshape(NLOC, OUT_W)
         for r in results], axis=0)                       # [N, K+2]
    shifted = raw[:, 0:K].astype(np.float64)
    negm = raw[:, K].astype(np.float64)
    ssum = raw[:, K + 1].astype(np.float64)
    lse = np.log(ssum)                                    # [N]
    resp = (shifted - lse[:, None]).astype(np.float32)    # log_resp_norm [N, K]
    ll = (lse - negm).astype(np.float32)                  # log_likelihood [N]
    return resp, ll


def kernel(X, log_pi, mu, Lambda, log_psi):
    nc = _get_program()
    in_maps = make_in_maps(X, log_pi, mu, Lambda, log_psi)
    res = run_bass_kernel_spmd(nc, in_maps, core_ids=list(range(N_CORES)))

    return finish_outputs(res.results)


if __name__ == "__main__":
    rng = np.random.default_rng(0)
    inputs = {
        "X": rng.standard_normal((N, D)).astype(np.float32),
        "log_pi": np.full((K,), -np.log(K), np.float32),
        "mu": (0.1 * rng.standard_normal((K, D))).astype(np.float32),
        "Lambda": (0.1 * rng.standard_normal((K, D, Q))).astype(np.float32),
        "log_psi": (np.log(0.01) + 0.1 * rng.standard_normal((K, D))).astype(np.float32),
    }
    resp, ll = kernel(**inputs)
    print("resp", resp.shape, resp.dtype, "ll", ll.shape, ll.dtype)


# revision 54
# speedup vs baseline: 1.3754x; 1.0026x over previous
"""MFA E-step kernel for Trainium2 (8 NeuronCores, data-parallel over N).

Math: the reference builds C_k = Lambda_k Lambda_k^T + diag(psi_k) in [K,D,D],
Cholesky-factors it and does a triangular solve per component (~17 GFLOP).
Since C_k is diagonal-plus-rank-Q we use the Woodbury identity and the matrix
determinant lemma instead:

  C^-1 = A^-1 - A^-1 L B^-1 L^T A^-1,  B = I_Q + L^T A^-1 L,  A = diag(a)
  log|C| = log|B| + sum log a

With B = R R^T (Cholesky, [Q,Q]=16x16 - tiny, done on host) and
G = A^-1 L R^-T  [D,Q], the per-sample work reduces to

  log_resp(k, x) = -0.5 * sum_d x^2 inv_a + (inv_a*mu).x + 0.5*||G^T x - g||^2 + C_k

i.e. everything n-dependent is matmuls against X with contraction over D,
plus an elementwise square. All of that accumulates into ONE PSUM tile
[K=32, n] per core on the tensor engine. The non-matmul device steps are the
squares (scalar engine; (t-g)^2 in one instruction via per-partition bias),
a PE transpose into one PSUM bank, and the max/exp/sum of the logsumexp.
The final scalar normalization (log of the [N]-vector of exp-sums and the
broadcast subtract) happens on host during the unshard/gather step, in
float64.

Matmul precision modes (MFA_MODE):
  split (default): bf16 hi/lo 3-term products, A.B ~= Ahi.Bhi+Ahi.Blo+Alo.Bhi
        at 1 cyc/row on the PE (~2x faster than fp32's 4 cyc/row) with
        ~2^-17 per-product error - comfortably inside the fp32 envelope.
  fp32: exact fp32 (walrus lowers to 2 half-speed passes each).
  f32r: single-pass TF32-like (fast but ~2.4e-4 rel error).

Sharding: X is split along N across the 8 cores (512 rows each); the small
component parameters are replicated. No collectives needed.

I/O is packed into few large DMAs per core - the Tile runtime's
end-of-kernel drain scales with DMA queue traffic.
"""

import os

import ml_dtypes
import numpy as np

import concourse.mybir as mybir
import concourse.tile as tile
from concourse import bacc
from concourse.bass_utils import run_bass_kernel_spmd

K, D, Q, N = 32, 256, 16, 4096
N_CORES = 8
NLOC = N // N_CORES          # 512 rows of X per core
KQ = K * Q                   # 512
LOG2PI = float(np.log(2.0 * np.pi))
FP = mybir.dt.float32
BF = mybir.dt.bfloat16

MODE = os.environ.get("MFA_MODE", "split")
assert MODE in ("split", "fp32", "f32r")
MM = mybir.dt.float32r if MODE == "f32r" else mybir.dt.float32
WARMUP_MM = int(os.environ.get("MFA_WARMUP", "2"))

# fp32/f32r packed input: [xt | gs | w12 | cst];  split mode: [xt | cst] fp32
IN_XT = 0
CST_GNEG = 0                  # 4 cols:   -g per kq-tile
CST_ONES = 4                  # 128 cols: 4 blocks of [128, 32] 0.5-valued
CST_ID = CST_ONES + 128       # 32 cols:  identity (rows 0:32)
CST_CK = CST_ID + 32          # 1 col:    per-component constant (rows 0:32)
CST_W = CST_CK + 1            # 165
if MODE == "split":
    IN_CST = NLOC
    # bf16 param block [gs_hi | gs_lo | w2pack | w1pack | ones] is packed
    # INSIDE the fp32 input tensor (2 bf16 per fp32 slot, bitcast on SBUF)
    # so each chunk is a single DMA - the Tile end-of-kernel drain scales
    # with DMA count.  w2pack = [W2hi|W2lo], w1pack = [W1hi|W1lo].
    INB_GSHI = 0
    INB_GSLO = KQ
    INB_W2P = 2 * KQ
    INB_W1P = 2 * KQ + 64
    INB_ONES = 2 * KQ + 128
    INB_W = INB_ONES + 128            # 1280 bf16 = 640 fp32 cols
    IN_BF = IN_CST + CST_W            # fp32 col where the bf16 block starts
    IN_W = IN_BF + INB_W // 2
else:
    IN_GS = NLOC
    IN_W12 = IN_GS + KQ
    IN_CST = IN_W12 + 64
    IN_W = IN_CST + CST_W

OUT_W = K + 2                 # [shifted(32) | -max | sum_exp]


def _fp(ap):
    """View an MM-dtype AP as plain float32 for non-matmul consumers."""
    return ap.bitcast(FP) if MODE == "f32r" else ap


def _build_program():
    nc = bacc.Bacc("TRN2", target_bir_lowering=False)

    inp = nc.dram_tensor("inp", [D, IN_W], MM, kind="ExternalInput")
    # output stays in SBUF-tile layout [128, j*OUT_W + k] so the store DMA is
    # fully contiguous (a scattered (j p) k -> p j k pattern measured ~2.6us
    # for 68KB; contiguous is ~0.4us) - rows are reordered on host
    out = nc.dram_tensor("out", [128, 4 * OUT_W], FP, kind="ExternalOutput")

    with tile.TileContext(nc) as tc:
        with (
            tc.tile_pool(name="data", bufs=1) as dpool,
            tc.tile_pool(name="sq", bufs=1) as spool,
            tc.tile_pool(name="small", bufs=2) as vpool,
            tc.tile_pool(name="warm", bufs=1) as wpool,
            tc.tile_pool(name="ttps", bufs=1 if MODE == "split" else 2,
                         space="PSUM") as ttpool,
            tc.tile_pool(name="rps", bufs=1, space="PSUM") as rpool,
            tc.tile_pool(name="tps", bufs=1, space="PSUM") as tpool,
            tc.tile_pool(name="wps", bufs=1, space="PSUM") as wpspool,
        ):
            # ---- PE warm-up: keep the tensor engine busy through the DMA
            # phase so HAM un-throttles (1.2 -> 2.4 GHz) before real work.
            if WARMUP_MM:
                wsrc = wpool.tile([128, NLOC], FP, tag="wsrc")
                nc.vector.memset(wsrc[:], 0.0)
                wps = wpspool.tile([128, NLOC], FP, tag="wps")
                for i in range(WARMUP_MM):
                    nc.tensor.matmul(wps[:], wsrc[:, 0:128], wsrc[:],
                                     start=(i == 0), stop=(i == WARMUP_MM - 1))

            # ---- loads: one packed DMA per chunk per dram tensor, split
            # across the two HWDGE issuing engines (Sync / Scalar) ----
            # one packed DMA per chunk, one per HWDGE issuing engine
            in_t = []
            for c in range(2):
                t = dpool.tile([128, IN_W], MM, tag=f"in{c}")
                eng = nc.sync if c == 0 else nc.scalar
                eng.dma_start(t[:], inp[c * 128:(c + 1) * 128, :])
                in_t.append(t)
            cst_t = in_t[0][:, IN_CST:IN_CST + CST_W]
            if MODE == "split":
                # X ships pre-split as bf16 [xhi | xlo] pairs (same bytes as
                # fp32): the tensor engine is ready at transfer-done instead
                # of transfer + on-device cast chain
                xb = [in_t[c][:, IN_XT:IN_XT + NLOC].bitcast(BF)
                      for c in range(2)]
                x_s = [(xb[c][:, 0:NLOC], xb[c][:, NLOC:2 * NLOC])
                       for c in range(2)]
                inb_t = [in_t[c][:, IN_BF:IN_BF + INB_W // 2].bitcast(BF)
                         for c in range(2)]
            else:
                xt_t = [in_t[c][:, IN_XT:IN_XT + NLOC] for c in range(2)]
            if MODE == "split":
                gs_hi = [inb_t[c][:, INB_GSHI:INB_GSHI + KQ] for c in range(2)]
                gs_lo = [inb_t[c][:, INB_GSLO:INB_GSLO + KQ] for c in range(2)]
                w2p = [inb_t[c][:, INB_W2P:INB_W2P + 64] for c in range(2)]
                w1p = [inb_t[c][:, INB_W1P:INB_W1P + 64] for c in range(2)]
                ones_t = [inb_t[0][:, INB_ONES + 32 * t:INB_ONES + 32 * (t + 1)]
                          for t in range(4)]
            else:
                gs_t = [in_t[c][:, IN_GS:IN_GS + KQ] for c in range(2)]
                w12_t = [in_t[c][:, IN_W12:IN_W12 + 64] for c in range(2)]
                ones_t = [cst_t[:, CST_ONES + 32 * t:CST_ONES + 32 * (t + 1)]
                          for t in range(4)]

            def hi_lo(src_fp, tag, hi_eng=None):
                """bf16 split of a [128, NLOC] fp32 AP: hi = bf16(x) on
                hi_eng (scalar/gpsimd - spreads cast load off the critical
                engine), lo = bf16(x - hi) on the vector engine."""
                hi = spool.tile([128, NLOC], BF, tag=f"{tag}hi")
                if hi_eng is None:
                    nc.scalar.copy(hi[:], src_fp)
                else:
                    hi_eng.tensor_copy(hi[:], src_fp)
                lo = spool.tile([128, NLOC], BF, tag=f"{tag}lo")
                nc.vector.tensor_tensor(lo[:], src_fp, hi[:],
                                        op=mybir.AluOpType.subtract)
                return hi, lo

            # ---- x^2 (scalar engine), bf16 splits (vector engine) ----
            xsq_t, xsq_s = [], []
            for c in range(2):
                xs = spool.tile([128, NLOC], MM, tag=f"xsq{c}")
                if MODE == "split":
                    # reconstruct fp32 x = xhi + xlo for the squaring path
                    xf = spool.tile([128, NLOC], FP, tag=f"xf{c}")
                    nc.vector.tensor_add(xf[:], x_s[c][0], x_s[c][1])
                    nc.scalar.square(xs[:], xf[:])
                    xsq_s.append(hi_lo(_fp(xs[:]), f"xsq{c}", nc.vector))
                else:
                    nc.scalar.square(xs[:], _fp(xt_t[c]))
                xsq_t.append(xs)

            # ---- T = G^T X^T  [KQ, NLOC] in 4 partition tiles; S = (T-g)^2 ----
            s_t, s_s = [], []
            if MODE == "split":
                # All 4 tile accumulation groups open at once (4 PSUM banks);
                # terms emitted in operand-readiness order: everything that
                # needs only the hi cast of a chunk before anything needing
                # its lo cast, chunk 0 before chunk 1.
                tts = [ttpool.tile([128, NLOC], FP, tag=f"tt{t}", name=f"tt{t}")
                       for t in range(4)]
                for ci, c in enumerate(range(2)):
                    for cls in range(3):      # 0: ghi.xhi, 1: glo.xhi, 2: ghi.xlo
                        for t in range(4):
                            ghi = gs_hi[c][:, t * 128:(t + 1) * 128]
                            glo = gs_lo[c][:, t * 128:(t + 1) * 128]
                            lh, rh = [(ghi, x_s[c][0]), (glo, x_s[c][0]),
                                      (ghi, x_s[c][1])][cls]
                            nc.tensor.matmul(tts[t][:], lh, rh,
                                             start=(ci == 0 and cls == 0),
                                             stop=(ci == 1 and cls == 2))
                for t in range(4):
                    s = spool.tile([128, NLOC], MM, tag=f"s{t}")
                    nc.scalar.activation(
                        s[:], tts[t][:], mybir.ActivationFunctionType.Square,
                        bias=_fp(cst_t[:, CST_GNEG + t:CST_GNEG + t + 1]),
                        scale=1.0,
                    )
                    s_t.append(s)
                    s_s.append(hi_lo(_fp(s[:]), f"s{t}", nc.vector))
            else:
                for t in range(4):
                    tt = ttpool.tile([128, NLOC], FP, tag="tt")
                    for c in range(2):
                        nc.tensor.matmul(tt[:], gs_t[c][:, t * 128:(t + 1) * 128],
                                         xt_t[c], start=(c == 0), stop=(c == 1))
                    s = spool.tile([128, NLOC], MM, tag=f"s{t}")
                    nc.scalar.activation(
                        s[:], tt[:], mybir.ActivationFunctionType.Square,
                        bias=_fp(cst_t[:, CST_GNEG + t:CST_GNEG + t + 1]),
                        scale=1.0,
                    )
                    s_t.append(s)

            # ---- single PSUM accumulation:  R = -0.5*P + U + 0.5*corr ----
            # split mode: 64-row psum; the hi-stationary terms land in rows
            # 0:32 and the lo-stationary terms in rows 32:64 (64-wide packed
            # stationaries make each 512-col moving pass do double duty);
            # the rs step sums the halves.
            rs = spool.tile([K, NLOC], FP, tag="rs")
            if MODE == "split":
                r_ps = rpool.tile([K, NLOC], FP, tag="r")
                racc = []  # (lhsT, rhs)
                for c in range(2):
                    racc += [(w2p[c][:, 0:K], x_s[c][0]),
                             (w2p[c][:, 0:K], x_s[c][1]),
                             (w2p[c][:, K:64], x_s[c][0]),
                             (w1p[c][:, 0:K], xsq_s[c][0][:]),
                             (w1p[c][:, 0:K], xsq_s[c][1][:]),
                             (w1p[c][:, K:64], xsq_s[c][0][:])]
                for t in range(4):
                    # ones (0.5) is exact in bf16 -> 2-term split suffices
                    racc += [(ones_t[t], s_s[t][0][:]),
                             (ones_t[t], s_s[t][1][:])]
                for i, (lhsT, rhs) in enumerate(racc):
                    nc.tensor.matmul(r_ps[:], lhsT, rhs,
                                     start=(i == 0), stop=(i == len(racc) - 1))
                nc.vector.tensor_scalar(
                    rs[:], r_ps[:], _fp(cst_t[0:K, CST_CK:CST_CK + 1]), None,
                    op0=mybir.AluOpType.add,
                )
            else:
                r_ps = rpool.tile([K, NLOC], FP, tag="r")
                racc = []
                for c in range(2):
                    racc += [(w12_t[c][:, K:64], xt_t[c]),
                             (w12_t[c][:, 0:K], xsq_t[c][:])]
                for t in range(4):
                    racc.append((ones_t[t], s_t[t][:]))
                for i, (lhsT, rhs) in enumerate(racc):
                    nc.tensor.matmul(r_ps[:], lhsT, rhs,
                                     start=(i == 0), stop=(i == len(racc) - 1))
                nc.vector.tensor_scalar(
                    rs[:], r_ps[:], _fp(cst_t[0:K, CST_CK:CST_CK + 1]), None,
                    op0=mybir.AluOpType.add,
                )

            # ---- transpose all 4 n-tiles into ONE psum bank [128, 4*K] ----
            tp = tpool.tile([128, 4 * K], FP, tag="tp")
            ident = _fp(cst_t[0:K, CST_ID:CST_ID + K])
            for j in range(4):
                nc.tensor.transpose(
                    tp[:, j * K:(j + 1) * K], rs[:, j * 128:(j + 1) * 128], ident)
            tp3 = tp[:].rearrange("p (j k) -> p j k", k=K)    # [128, 4, K]

            # ---- batched max/exp/sum of the logsumexp; pack one out tile ----
            outt = spool.tile([128, 4 * OUT_W], FP, tag="outt")
            o3 = outt[:].rearrange("p (j k) -> p j k", k=OUT_W)
            negm = o3[:, :, K]                                # [128, 4]
            nc.vector.tensor_reduce(
                o3[:, :, K:K + 1], tp3, axis=mybir.AxisListType.X,
                op=mybir.AluOpType.max, negate=True,
            )
            sh3 = o3[:, :, 0:K]
            nc.vector.tensor_tensor(
                sh3, tp3, negm.broadcast_to([128, 4, K]),
                op=mybir.AluOpType.add,                       # t - max
            )
            e = spool.tile([128, 4 * K], FP, tag="e")
            nc.scalar.activation(
                e[:].rearrange("p (j k) -> p j k", k=K), sh3,
                mybir.ActivationFunctionType.Exp)
            nc.vector.tensor_reduce(
                o3[:, :, K + 1:K + 2], e[:].rearrange("p (j k) -> p j k", k=K),
                axis=mybir.AxisListType.X, op=mybir.AluOpType.add,
            )

            nc.sync.dma_start(out[:, :], outt[:])

    nc.finalize()
    return nc


_PROGRAM_CACHE = {}


def _get_program():
    if MODE not in _PROGRAM_CACHE:
        _PROGRAM_CACHE[MODE] = _build_program()
    return _PROGRAM_CACHE[MODE]


def _bf_split(A):
    """bf16 (hi, lo) split of a float64 array."""
    hi = A.astype(ml_dtypes.bfloat16)
    lo = (A - hi.astype(np.float64)).astype(ml_dtypes.bfloat16)
    return hi, lo


def _host_prep(X, log_pi, mu, Lambda, log_psi):
    """Tiny O(K*D*Q^2) parameter prep in float64 on host."""
    X = np.asarray(X, np.float64)
    log_pi = np.asarray(log_pi, np.float64)
    mu = np.asarray(mu, np.float64)
    Lam = np.asarray(Lambda, np.float64)
    log_psi = np.asarray(log_psi, np.float64)

    a = np.exp(log_psi) + 1e-6 + 1e-5                     # [K, D]
    inv_a = 1.0 / a
    AL = Lam * inv_a[:, :, None]                          # [K, D, Q]
    B = np.eye(Q)[None] + np.einsum('kdq,kde->kqe', Lam, AL)
    R = np.linalg.cholesky(B)                             # [K, Q, Q]
    logdet = 2.0 * np.sum(np.log(np.diagonal(R, axis1=1, axis2=2)), axis=1) \
        + np.sum(np.log(a), axis=1)                       # [K]
    G = np.linalg.solve(R, AL.transpose(0, 2, 1)).transpose(0, 2, 1)  # [K, D, Q]
    g = np.einsum('kdq,kd->kq', G, mu)                    # [K, Q]
    Ck = log_pi - 0.5 * (D * LOG2PI + logdet + np.sum(mu * mu * inv_a, axis=1))

    f = np.float32
    gsm = G.transpose(1, 0, 2).reshape(D, KQ)             # G as [D, k*Q+q]
    w12 = np.concatenate([-0.5 * inv_a.T, (inv_a * mu).T], axis=1)  # [D, 64]

    cstm = np.zeros((128, CST_W), f)
    # gneg col t, partition p  <-  -g_flat[t*128 + p]  (kq index = k*Q + q)
    cstm[:, CST_GNEG:CST_GNEG + 4] = (-g).reshape(4, 128).T
    onesm = np.zeros((128, 128), f)
    for t in range(4):
        for p in range(128):
            onesm[p, 32 * t + (t * 128 + p) // Q] = 0.5
    cstm[:, CST_ONES:CST_ONES + 128] = onesm
    cstm[0:K, CST_ID:CST_ID + K] = np.eye(K, dtype=f)
    cstm[0:K, CST_CK] = Ck.astype(f)
    xt_full = np.ascontiguousarray(X.T.astype(f))         # [D, N]

    if MODE == "split":
        xhi_full, xlo_full = _bf_split(X.T)               # bf16 [D, N] pair
        parb = np.zeros((D, INB_W), ml_dtypes.bfloat16)
        gh, gl = _bf_split(gsm)
        wh, wl = _bf_split(w12)
        parb[:, INB_GSHI:INB_GSHI + KQ] = gh
        parb[:, INB_GSLO:INB_GSLO + KQ] = gl
        # w2pack = [W2hi|W2lo], w1pack = [W1hi|W1lo]  (w12 = [W1 | W2])
        parb[:, INB_W2P:INB_W2P + K] = wh[:, K:64]
        parb[:, INB_W2P + K:INB_W2P + 64] = wl[:, K:64]
        parb[:, INB_W1P:INB_W1P + K] = wh[:, 0:K]
        parb[:, INB_W1P + K:INB_W1P + 64] = wl[:, 0:K]
        parb[0:128, INB_ONES:INB_ONES + 128] = onesm.astype(ml_dtypes.bfloat16)
        par = cstm                                        # [128, CST_W]
        return (xhi_full, xlo_full), par, parb
    else:
        par = np.zeros((D, IN_W - NLOC), f)               # [gs | w12 | cst]
        par[:, 0:KQ] = gsm
        par[:, KQ:KQ + 64] = w12
        par[0:128, KQ + 64:] = cstm
        return xt_full, par, None


def make_in_maps(X, log_pi, mu, Lambda, log_psi):
    xt_full, par, parb = _host_prep(X, log_pi, mu, Lambda, log_psi)
    in_maps = []
    for c in range(N_CORES):
        buf = np.zeros((D, IN_W), np.float32)
        if MODE == "split":
            # X as bf16 [xhi | xlo] pairs in the first NLOC fp32 slots
            xhi_full, xlo_full = xt_full
            bu16 = buf.view(np.uint16)
            bu16[:, 0:NLOC] = xhi_full[:, c * NLOC:(c + 1) * NLOC].view(np.uint16)
            bu16[:, NLOC:2 * NLOC] = xlo_full[:, c * NLOC:(c + 1) * NLOC].view(np.uint16)
            buf[0:128, IN_CST:IN_CST + CST_W] = par
            bu16[:, 2 * IN_BF:2 * IN_BF + INB_W] = parb.view(np.uint16)
        else:
            buf[:, 0:NLOC] = xt_full[:, c * NLOC:(c + 1) * NLOC]
            buf[:, NLOC:] = par
        in_maps.append({"inp": buf})
    return in_maps


def finish_outputs(results):
    """Gather per-core outputs; final scalar normalization in float64."""
    raw = np.concatenate(
        [r["out"].reshape(128, 4, OUT_W).transpose(1, 0, 2).reshape(NLOC, OUT_W)
         for r in results], axis=0)                       # [N, K+2]
    shifted = raw[:, 0:K].astype(np.float64)
    negm = raw[:, K].astype(np.float64)
    ssum = raw[:, K + 1].astype(np.float64)
    lse = np.log(ssum)                                    # [N]
    resp = (shifted - lse[:, None]).astype(np.float32)    # log_resp_norm [N, K]
    ll = (lse - negm).astype(np.float32)                  # log_likelihood [N]
    return resp, ll


def kernel(X, log_pi, mu, Lambda, log_psi):
    nc = _get_program()
    in_maps = make_in_maps(X, log_pi, mu, Lambda, log_psi)
    res = run_bass_kernel_spmd(nc, in_maps, core_ids=list(range(N_CORES)))

    return finish_outputs(res.results)


if __name__ == "__main__":
    rng = np.random.default_rng(0)
    inputs = {
        "X": rng.standard_normal((N, D)).astype(np.float32),
        "log_pi": np.full((K,), -np.log(K), np.float32),
        "mu": (0.1 * rng.standard_normal((K, D))).astype(np.float32),
        "Lambda": (0.1 * rng.standard_normal((K, D, Q))).astype(np.float32),
        "log_psi": (np.log(0.01) + 0.1 * rng.standard_normal((K, D))).astype(np.float32),
    }
    resp, ll = kernel(**inputs)
    print("resp", resp.shape, resp.dtype, "ll", ll.shape, ll.dtype)


# revision 58
# speedup vs baseline: 1.3815x; 1.0045x over previous
"""MFA E-step kernel for Trainium2 (8 NeuronCores, data-parallel over N).

Math: the reference builds C_k = Lambda_k Lambda_k^T + diag(psi_k) in [K,D,D],
Cholesky-factors it and does a triangular solve per component (~17 GFLOP).
Since C_k is diagonal-plus-rank-Q we use the Woodbury identity and the matrix
determinant lemma instead:

  C^-1 = A^-1 - A^-1 L B^-1 L^T A^-1,  B = I_Q + L^T A^-1 L,  A = diag(a)
  log|C| = log|B| + sum log a

With B = R R^T (Cholesky, [Q,Q]=16x16 - tiny, done on host) and
G = A^-1 L R^-T  [D,Q], the per-sample work reduces to

  log_resp(k, x) = -0.5 * sum_d x^2 inv_a + (inv_a*mu).x + 0.5*||G^T x - g||^2 + C_k

i.e. everything n-dependent is matmuls against X with contraction over D,
plus an elementwise square. All of that accumulates into ONE PSUM tile
[K=32, n] per core on the tensor engine. The non-matmul device steps are the
squares (scalar engine; (t-g)^2 in one instruction via per-partition bias),
a PE transpose into one PSUM bank, and the max/exp/sum of the logsumexp.
The final scalar normalization (log of the [N]-vector of exp-sums and the
broadcast subtract) happens on host during the unshard/gather step, in
float64.

Matmul precision modes (MFA_MODE):
  split (default): bf16 hi/lo 3-term products, A.B ~= Ahi.Bhi+Ahi.Blo+Alo.Bhi
        at 1 cyc/row on the PE (~2x faster than fp32's 4 cyc/row) with
        ~2^-17 per-product error - comfortably inside the fp32 envelope.
  fp32: exact fp32 (walrus lowers to 2 half-speed passes each).
  f32r: single-pass TF32-like (fast but ~2.4e-4 rel error).

Sharding: X is split along N across the 8 cores (512 rows each); the small
component parameters are replicated. No collectives needed.

I/O is packed into few large DMAs per core - the Tile runtime's
end-of-kernel drain scales with DMA queue traffic.
"""

import os

import ml_dtypes
import numpy as np

import concourse.mybir as mybir
import concourse.tile as tile
from concourse import bacc
from concourse.bass_utils import run_bass_kernel_spmd

K, D, Q, N = 32, 256, 16, 4096
N_CORES = 8
NLOC = N // N_CORES          # 512 rows of X per core
KQ = K * Q                   # 512
LOG2PI = float(np.log(2.0 * np.pi))
FP = mybir.dt.float32
BF = mybir.dt.bfloat16

MODE = os.environ.get("MFA_MODE", "split")
assert MODE in ("split", "fp32", "f32r")
MM = mybir.dt.float32r if MODE == "f32r" else mybir.dt.float32
WARMUP_MM = int(os.environ.get("MFA_WARMUP", "2"))

# fp32/f32r packed input: [xt | gs | w12 | cst];  split mode: [xt | cst] fp32
IN_XT = 0
CST_GNEG = 0                  # 4 cols:   -g per kq-tile
CST_ONES = 4                  # 128 cols: 4 blocks of [128, 32] 0.5-valued
CST_ID = CST_ONES + 128       # 32 cols:  identity (rows 0:32)
CST_CK = CST_ID + 32          # 1 col:    per-component constant (rows 0:32)
CST_W = CST_CK + 1            # 165
if MODE == "split":
    IN_CST = NLOC
    # bf16 param block [gs_hi | gs_lo | w2pack | w1pack | ones] is packed
    # INSIDE the fp32 input tensor (2 bf16 per fp32 slot, bitcast on SBUF)
    # so each chunk is a single DMA - the Tile end-of-kernel drain scales
    # with DMA count.  w2pack = [W2hi|W2lo], w1pack = [W1hi|W1lo].
    INB_GSHI = 0
    INB_GSLO = KQ
    INB_W2P = 2 * KQ
    INB_W1P = 2 * KQ + 64
    INB_ONES = 2 * KQ + 128
    INB_W = INB_ONES + 128            # 1280 bf16 = 640 fp32 cols
    IN_BF = IN_CST + CST_W            # fp32 col where the bf16 block starts
    IN_W = IN_BF + INB_W // 2
else:
    IN_GS = NLOC
    IN_W12 = IN_GS + KQ
    IN_CST = IN_W12 + 64
    IN_W = IN_CST + CST_W

OUT_W = K + 2                 # [shifted(32) | -max | sum_exp]


def _fp(ap):
    """View an MM-dtype AP as plain float32 for non-matmul consumers."""
    return ap.bitcast(FP) if MODE == "f32r" else ap


def _build_program():
    nc = bacc.Bacc("TRN2", target_bir_lowering=False)

    inp = nc.dram_tensor("inp", [D, IN_W], MM, kind="ExternalInput")
    # output stays in SBUF-tile layout [128, j*OUT_W + k] so the store DMA is
    # fully contiguous (a scattered (j p) k -> p j k pattern measured ~2.6us
    # for 68KB; contiguous is ~0.4us) - rows are reordered on host
    out = nc.dram_tensor("out", [128, 4 * OUT_W], FP, kind="ExternalOutput")

    with tile.TileContext(nc) as tc:
        with (
            tc.tile_pool(name="data", bufs=1) as dpool,
            tc.tile_pool(name="sq", bufs=1) as spool,
            tc.tile_pool(name="small", bufs=2) as vpool,
            tc.tile_pool(name="warm", bufs=1) as wpool,
            tc.tile_pool(name="ttps", bufs=1 if MODE == "split" else 2,
                         space="PSUM") as ttpool,
            tc.tile_pool(name="rps", bufs=1, space="PSUM") as rpool,
            tc.tile_pool(name="tps", bufs=1, space="PSUM") as tpool,
            tc.tile_pool(name="wps", bufs=1, space="PSUM") as wpspool,
        ):
            # ---- PE warm-up: keep the tensor engine busy through the DMA
            # phase so HAM un-throttles (1.2 -> 2.4 GHz) before real work.
            if WARMUP_MM:
                wsrc = wpool.tile([128, NLOC], FP, tag="wsrc")
                nc.vector.memset(wsrc[:], 0.0)
                wps = wpspool.tile([128, NLOC], FP, tag="wps")
                for i in range(WARMUP_MM):
                    nc.tensor.matmul(wps[:], wsrc[:, 0:128], wsrc[:],
                                     start=(i == 0), stop=(i == WARMUP_MM - 1))

            # ---- loads: one packed DMA per chunk per dram tensor, split
            # across the two HWDGE issuing engines (Sync / Scalar) ----
            # one packed DMA per chunk, one per HWDGE issuing engine
            in_t = []
            for c in range(2):
                t = dpool.tile([128, IN_W], MM, tag=f"in{c}")
                eng = nc.sync if c == 0 else nc.scalar
                eng.dma_start(t[:], inp[c * 128:(c + 1) * 128, :])
                in_t.append(t)
            cst_t = in_t[0][:, IN_CST:IN_CST + CST_W]
            if MODE == "split":
                # X ships pre-split as bf16 [xhi | xlo] pairs (same bytes as
                # fp32): the tensor engine is ready at transfer-done instead
                # of transfer + on-device cast chain
                xb = [in_t[c][:, IN_XT:IN_XT + NLOC].bitcast(BF)
                      for c in range(2)]
                x_s = [(xb[c][:, 0:NLOC], xb[c][:, NLOC:2 * NLOC])
                       for c in range(2)]
                inb_t = [in_t[c][:, IN_BF:IN_BF + INB_W // 2].bitcast(BF)
                         for c in range(2)]
            else:
                xt_t = [in_t[c][:, IN_XT:IN_XT + NLOC] for c in range(2)]
            if MODE == "split":
                gs_hi = [inb_t[c][:, INB_GSHI:INB_GSHI + KQ] for c in range(2)]
                gs_lo = [inb_t[c][:, INB_GSLO:INB_GSLO + KQ] for c in range(2)]
                w2p = [inb_t[c][:, INB_W2P:INB_W2P + 64] for c in range(2)]
                w1p = [inb_t[c][:, INB_W1P:INB_W1P + 64] for c in range(2)]
                ones_t = [inb_t[0][:, INB_ONES + 32 * t:INB_ONES + 32 * (t + 1)]
                          for t in range(4)]
            else:
                gs_t = [in_t[c][:, IN_GS:IN_GS + KQ] for c in range(2)]
                w12_t = [in_t[c][:, IN_W12:IN_W12 + 64] for c in range(2)]
                ones_t = [cst_t[:, CST_ONES + 32 * t:CST_ONES + 32 * (t + 1)]
                          for t in range(4)]

            def hi_lo(src_fp, tag, hi_eng=None):
                """bf16 split of a [128, NLOC] fp32 AP: hi = bf16(x) on
                hi_eng (scalar/gpsimd - spreads cast load off the critical
                engine), lo = bf16(x - hi) on the vector engine."""
                hi = spool.tile([128, NLOC], BF, tag=f"{tag}hi")
                if hi_eng is None:
                    nc.scalar.copy(hi[:], src_fp)
                else:
                    hi_eng.tensor_copy(hi[:], src_fp)
                lo = spool.tile([128, NLOC], BF, tag=f"{tag}lo")
                nc.vector.tensor_tensor(lo[:], src_fp, hi[:],
                                        op=mybir.AluOpType.subtract)
                return hi, lo

            # ---- x^2 (scalar engine), bf16 splits (vector engine) ----
            xsq_t, xsq_s = [], []
            for c in range(2):
                xs = spool.tile([128, NLOC], MM, tag=f"xsq{c}")
                if MODE == "split":
                    # reconstruct fp32 x = xhi + xlo for the squaring path
                    xf = spool.tile([128, NLOC], FP, tag=f"xf{c}")
                    nc.vector.tensor_add(xf[:], x_s[c][0], x_s[c][1])
                    nc.scalar.square(xs[:], xf[:])
                    xsq_s.append(hi_lo(_fp(xs[:]), f"xsq{c}", nc.vector))
                else:
                    nc.scalar.square(xs[:], _fp(xt_t[c]))
                xsq_t.append(xs)

            # ---- T = G^T X^T  [KQ, NLOC] in 4 partition tiles; S = (T-g)^2 ----
            s_t, s_s = [], []
            if MODE == "split":
                # All 4 tile accumulation groups open at once (4 PSUM banks);
                # terms emitted in operand-readiness order: everything that
                # needs only the hi cast of a chunk before anything needing
                # its lo cast, chunk 0 before chunk 1.
                tts = [ttpool.tile([128, NLOC], FP, tag=f"tt{t}", name=f"tt{t}")
                       for t in range(4)]
                for ci, c in enumerate(range(2)):
                    for cls in range(3):      # 0: ghi.xhi, 1: glo.xhi, 2: ghi.xlo
                        for t in range(4):
                            ghi = gs_hi[c][:, t * 128:(t + 1) * 128]
                            glo = gs_lo[c][:, t * 128:(t + 1) * 128]
                            lh, rh = [(ghi, x_s[c][0]), (glo, x_s[c][0]),
                                      (ghi, x_s[c][1])][cls]
                            nc.tensor.matmul(tts[t][:], lh, rh,
                                             start=(ci == 0 and cls == 0),
                                             stop=(ci == 1 and cls == 2))
                for t in range(4):
                    s = spool.tile([128, NLOC], MM, tag=f"s{t}")
                    nc.scalar.activation(
                        s[:], tts[t][:], mybir.ActivationFunctionType.Square,
                        bias=_fp(cst_t[:, CST_GNEG + t:CST_GNEG + t + 1]),
                        scale=1.0,
                    )
                    s_t.append(s)
                    s_s.append(hi_lo(_fp(s[:]), f"s{t}", nc.vector))
            else:
                for t in range(4):
                    tt = ttpool.tile([128, NLOC], FP, tag="tt")
                    for c in range(2):
                        nc.tensor.matmul(tt[:], gs_t[c][:, t * 128:(t + 1) * 128],
                                         xt_t[c], start=(c == 0), stop=(c == 1))
                    s = spool.tile([128, NLOC], MM, tag=f"s{t}")
                    nc.scalar.activation(
                        s[:], tt[:], mybir.ActivationFunctionType.Square,
                        bias=_fp(cst_t[:, CST_GNEG + t:CST_GNEG + t + 1]),
                        scale=1.0,
                    )
                    s_t.append(s)

            # ---- single PSUM accumulation:  R = -0.5*P + U + 0.5*corr ----
            # split mode: 64-row psum; the hi-stationary terms land in rows
            # 0:32 and the lo-stationary terms in rows 32:64 (64-wide packed
            # stationaries make each 512-col moving pass do double duty);
            # the rs step sums the halves.
            rs = spool.tile([K, NLOC], FP, tag="rs")
            if MODE == "split":
                r_ps = rpool.tile([K, NLOC], FP, tag="r")
                racc = []  # (lhsT, rhs)
                for c in range(2):
                    racc += [(w2p[c][:, 0:K], x_s[c][0]),
                             (w2p[c][:, 0:K], x_s[c][1]),
                             (w2p[c][:, K:64], x_s[c][0]),
                             (w1p[c][:, 0:K], xsq_s[c][0][:]),
                             (w1p[c][:, 0:K], xsq_s[c][1][:]),
                             (w1p[c][:, K:64], xsq_s[c][0][:])]
                for t in range(4):
                    # ones (0.5) is exact in bf16 -> 2-term split suffices
                    racc += [(ones_t[t], s_s[t][0][:]),
                             (ones_t[t], s_s[t][1][:])]
                for i, (lhsT, rhs) in enumerate(racc):
                    nc.tensor.matmul(r_ps[:], lhsT, rhs,
                                     start=(i == 0), stop=(i == len(racc) - 1))
                nc.vector.tensor_scalar(
                    rs[:], r_ps[:], _fp(cst_t[0:K, CST_CK:CST_CK + 1]), None,
                    op0=mybir.AluOpType.add,
                )
            else:
                r_ps = rpool.tile([K, NLOC], FP, tag="r")
                racc = []
                for c in range(2):
                    racc += [(w12_t[c][:, K:64], xt_t[c]),
                             (w12_t[c][:, 0:K], xsq_t[c][:])]
                for t in range(4):
                    racc.append((ones_t[t], s_t[t][:]))
                for i, (lhsT, rhs) in enumerate(racc):
                    nc.tensor.matmul(r_ps[:], lhsT, rhs,
                                     start=(i == 0), stop=(i == len(racc) - 1))
                nc.vector.tensor_scalar(
                    rs[:], r_ps[:], _fp(cst_t[0:K, CST_CK:CST_CK + 1]), None,
                    op0=mybir.AluOpType.add,
                )

            # ---- transpose all 4 n-tiles into ONE psum bank [128, 4*K] ----
            tp = tpool.tile([128, 4 * K], FP, tag="tp")
            ident = _fp(cst_t[0:K, CST_ID:CST_ID + K])
            for j in range(4):
                nc.tensor.transpose(
                    tp[:, j * K:(j + 1) * K], rs[:, j * 128:(j + 1) * 128], ident)
            tp3 = tp[:].rearrange("p (j k) -> p j k", k=K)    # [128, 4, K]

            # ---- batched max/exp/sum of the logsumexp; pack one out tile ----
            outt = spool.tile([128, 4 * OUT_W], FP, tag="outt")
            o3 = outt[:].rearrange("p (j k) -> p j k", k=OUT_W)
            negm = o3[:, :, K]                                # [128, 4]
            nc.vector.tensor_reduce(
                o3[:, :, K:K + 1], tp3, axis=mybir.AxisListType.X,
                op=mybir.AluOpType.max, negate=True,
            )
            sh3 = o3[:, :, 0:K]
            nc.vector.tensor_tensor(
                sh3, tp3, negm.broadcast_to([128, 4, K]),
                op=mybir.AluOpType.add,                       # t - max
            )
            e = spool.tile([128, 4 * K], FP, tag="e")
            nc.scalar.activation(
                e[:].rearrange("p (j k) -> p j k", k=K), sh3,
                mybir.ActivationFunctionType.Exp)
            nc.vector.tensor_reduce(
                o3[:, :, K + 1:K + 2], e[:].rearrange("p (j k) -> p j k", k=K),
                axis=mybir.AxisListType.X, op=mybir.AluOpType.add,
            )

            nc.sync.dma_start(out[:, :], outt[:])

    nc.finalize()
    return nc


_PROGRAM_CACHE = {}


def _get_program():
    if MODE not in _PROGRAM_CACHE:
        _PROGRAM_CACHE[MODE] = _build_program()
    return _PROGRAM_CACHE[MODE]


def _bf_split(A):
    """bf16 (hi, lo) split of a float64 array."""
    hi = A.astype(ml_dtypes.bfloat16)
    lo = (A - hi.astype(np.float64)).astype(ml_dtypes.bfloat16)
    return hi, lo


def _host_prep(X, log_pi, mu, Lambda, log_psi):
    """Tiny O(K*D*Q^2) parameter prep in float64 on host."""
    X = np.asarray(X, np.float64)
    log_pi = np.asarray(log_pi, np.float64)
    mu = np.asarray(mu, np.float64)
    Lam = np.asarray(Lambda, np.float64)
    log_psi = np.asarray(log_psi, np.float64)

    a = np.exp(log_psi) + 1e-6 + 1e-5                     # [K, D]
    inv_a = 1.0 / a
    AL = Lam * inv_a[:, :, None]                          # [K, D, Q]
    B = np.eye(Q)[None] + np.einsum('kdq,kde->kqe', Lam, AL)
    R = np.linalg.cholesky(B)                             # [K, Q, Q]
    logdet = 2.0 * np.sum(np.log(np.diagonal(R, axis1=1, axis2=2)), axis=1) \
        + np.sum(np.log(a), axis=1)                       # [K]
    G = np.linalg.solve(R, AL.transpose(0, 2, 1)).transpose(0, 2, 1)  # [K, D, Q]
    g = np.einsum('kdq,kd->kq', G, mu)                    # [K, Q]
    Ck = log_pi - 0.5 * (D * LOG2PI + logdet + np.sum(mu * mu * inv_a, axis=1))

    f = np.float32
    gsm = G.transpose(1, 0, 2).reshape(D, KQ)             # G as [D, k*Q+q]
    w12 = np.concatenate([-0.5 * inv_a.T, (inv_a * mu).T], axis=1)  # [D, 64]

    cstm = np.zeros((128, CST_W), f)
    # gneg col t, partition p  <-  -g_flat[t*128 + p]  (kq index = k*Q + q)
    cstm[:, CST_GNEG:CST_GNEG + 4] = (-g).reshape(4, 128).T
    onesm = np.zeros((128, 128), f)
    for t in range(4):
        for p in range(128):
            onesm[p, 32 * t + (t * 128 + p) // Q] = 0.5
    cstm[:, CST_ONES:CST_ONES + 128] = onesm
    cstm[0:K, CST_ID:CST_ID + K] = np.eye(K, dtype=f)
    cstm[0:K, CST_CK] = Ck.astype(f)
    xt_full = np.ascontiguousarray(X.T.astype(f))         # [D, N]

    if MODE == "split":
        xhi_full, xlo_full = _bf_split(X.T)               # bf16 [D, N] pair
        parb = np.zeros((D, INB_W), ml_dtypes.bfloat16)
        gh, gl = _bf_split(gsm)
        wh, wl = _bf_split(w12)
        parb[:, INB_GSHI:INB_GSHI + KQ] = gh
        parb[:, INB_GSLO:INB_GSLO + KQ] = gl
        # w2pack = [W2hi|W2lo], w1pack = [W1hi|W1lo]  (w12 = [W1 | W2])
        parb[:, INB_W2P:INB_W2P + K] = wh[:, K:64]
        parb[:, INB_W2P + K:INB_W2P + 64] = wl[:, K:64]
        parb[:, INB_W1P:INB_W1P + K] = wh[:, 0:K]
        parb[:, INB_W1P + K:INB_W1P + 64] = wl[:, 0:K]
        parb[0:128, INB_ONES:INB_ONES + 128] = onesm.astype(ml_dtypes.bfloat16)
        par = cstm                                        # [128, CST_W]
        return (xhi_full, xlo_full), par, parb
    else:
        par = np.zeros((D, IN_W - NLOC), f)               # [gs | w12 | cst]
        par[:, 0:KQ] = gsm
        par[:, KQ:KQ + 64] = w12
        par[0:128, KQ + 64:] = cstm
        return xt_full, par, None


def make_in_maps(X, log_pi, mu, Lambda, log_psi):
    xt_full, par, parb = _host_prep(X, log_pi, mu, Lambda, log_psi)
    in_maps = []
    for c in range(N_CORES):
        buf = np.zeros((D, IN_W), np.float32)
        if MODE == "split":
            # X as bf16 [xhi | xlo] pairs in the first NLOC fp32 slots
            xhi_full, xlo_full = xt_full
            bu16 = buf.view(np.uint16)
            bu16[:, 0:NLOC] = xhi_full[:, c * NLOC:(c + 1) * NLOC].view(np.uint16)
            bu16[:, NLOC:2 * NLOC] = xlo_full[:, c * NLOC:(c + 1) * NLOC].view(np.uint16)
            buf[0:128, IN_CST:IN_CST + CST_W] = par
            bu16[:, 2 * IN_BF:2 * IN_BF + INB_W] = parb.view(np.uint16)
        else:
            buf[:, 0:NLOC] = xt_full[:, c * NLOC:(c + 1) * NLOC]
            buf[:, NLOC:] = par
        in_maps.append({"inp": buf})
    return in_maps


def finish_outputs(results):
    """Gather per-core outputs; final scalar normalization in float64."""
    raw = np.concatenate(
        [r["out"].reshape(128, 4, OUT_W).transpose(1, 0, 2).reshape(NLOC, OUT_W)
         for r in results], axis=0)                       # [N, K+2]
    shifted = raw[:, 0:K].astype(np.float64)
    negm = raw[:, K].astype(np.float64)
    ssum = raw[:, K + 1].astype(np.float64)
    lse = np.log(ssum)                                    # [N]
    resp = (shifted - lse[:, None]).astype(np.float32)    # log_resp_norm [N, K]
    ll = (lse - negm).astype(np.float32)                  # log_likelihood [N]
    return resp, ll


def kernel(X, log_pi, mu, Lambda, log_psi):
    nc = _get_program()
    in_maps = make_in_maps(X, log_pi, mu, Lambda, log_psi)
    res = run_bass_kernel_spmd(nc, in_maps, core_ids=list(range(N_CORES)))

    return finish_outputs(res.results)


if __name__ == "__main__":
    rng = np.random.default_rng(0)
    inputs = {
        "X": rng.standard_normal((N, D)).astype(np.float32),
        "log_pi": np.full((K,), -np.log(K), np.float32),
        "mu": (0.1 * rng.standard_normal((K, D))).astype(np.float32),
        "Lambda": (0.1 * rng.standard_normal((K, D, Q))).astype(np.float32),
        "log_psi": (np.log(0.01) + 0.1 * rng.standard_normal((K, D))).astype(np.float32),
    }
    resp, ll = kernel(**inputs)
    print("resp", resp.shape, resp.dtype, "ll", ll.shape, ll.dtype)
